# revision 1
# baseline (speedup 1.0000x reference)
"""3-layer GCN (PyG GCNConv semantics) on 8 Trainium2 NeuronCores.

Strategy (graph/data parallel, dst-sharded):
  - Host: degrees + symmetric norm, node permutation (degree-sorted within
    each core's dst shard), layer-1 table g1 = (dinv*x1) @ W11, edge lists
    per core sorted by (src-region, dst-block), padded to a COMMON tile
    structure across cores (SPMD: one program, per-core data).
  - Device per layer: dma_gather streams g[src] rows (64-f32 padded rows,
    int16 region-relative indices, 4 SWDGE queues) into SBUF; VectorE builds
    per-tile one-hot selection matrices (is_equal vs an iota constant); PE
    computes sel.T @ msg per tile, accumulating in PSUM per (region, block)
    group; groups are combined into an SBUF accumulator (segment-sum by
    destination). Epilogue per 128-node block: scale by dinv, +bias, relu,
    PE transpose + matmul with next W, scale by dinv, write next-layer
    table shard. AllGather exchanges table shards between layers.
  - Final layer: y = x4 @ fc_w + fc_b per block.
"""

import numpy as np

P = 128
ELEM = 64          # gathered row: 32 real f32 + 32 pad (256B descriptor)
D = 32             # feature width
MAX_CALL = 8192    # max indices per dma_gather call
NCORES = 8
NREG = 4           # src index regions (int16 reach)


# ----------------------------------------------------------------- host side

def _preprocess(x1, edge_index):
    N = x1.shape[0]
    assert N % NCORES == 0
    NSH = N // NCORES                       # dst nodes per core
    NLOC = ((NSH + P - 1) // P) * P         # padded to blocks of 128
    NB = NLOC // P
    TSH = NLOC + 16                         # table rows per rank (16 zero pad)
    TBL = TSH * NCORES                      # total table rows
    assert TBL % NREG == 0
    RSPAN = TBL // NREG                     # table rows per region
    assert RSPAN <= 32767 and RSPAN == 2 * TSH

    src = np.asarray(edge_index[0], dtype=np.int64)
    dst = np.asarray(edge_index[1], dtype=np.int64)
    deg = np.bincount(dst, minlength=N).astype(np.float64) + 1.0
    dinv = (1.0 / np.sqrt(deg)).astype(np.float32)

    core_of = dst // NSH

    # per-core permutation (in-degree desc within shard) and global->table map
    perms, invperms = [], []
    g2t = np.empty(N, np.int64)
    dcnt_all = np.bincount(dst, minlength=N) + 1   # incl self-loop
    for c in range(NCORES):
        cnt = dcnt_all[c * NSH:(c + 1) * NSH]
        perm = np.argsort(-cnt, kind="stable")     # slot -> local node
        inv = np.empty(NSH, np.int64)
        inv[perm] = np.arange(NSH)
        perms.append(perm)
        invperms.append(inv)
        g2t[c * NSH:(c + 1) * NSH] = c * TSH + inv

    # per-core edge lists, incl self-loops; keyed by (region, dst-block)
    per_core = []
    for c in range(NCORES):
        m = core_of == c
        s_c = src[m]
        dslot = invperms[c][dst[m] - c * NSH]
        sl_s = np.arange(c * NSH, (c + 1) * NSH)
        sl_d = invperms[c]
        s_all = np.concatenate([s_c, sl_s])
        d_all = np.concatenate([dslot, sl_d])
        stid = g2t[s_all]
        reg = stid // RSPAN
        blk = d_all // P
        per_core.append((stid, d_all, reg, blk))

    # common tile structure: tiles_rb[r, b] = max over cores
    counts = np.zeros((NCORES, NREG, NB), np.int64)
    for c in range(NCORES):
        _, _, reg, blk = per_core[c]
        np.add.at(counts[c], (reg, blk), 1)
    tiles_rb = np.maximum((counts.max(axis=0) + P - 1) // P, 1)  # [NREG, NB]

    # ordered tile list (region-major, block order) + group boundaries
    tile_meta = []        # (region, block, group_first, group_last)
    for r in range(NREG):
        for b in range(NB):
            nt = int(tiles_rb[r, b])
            for i in range(nt):
                tile_meta.append((r, b, i == 0, i == nt - 1))
    T = len(tile_meta)

    # gather calls: chunk tile stream, never crossing region boundaries
    calls = []            # (region, tile_start, ntiles)
    t0 = 0
    while t0 < T:
        r = tile_meta[t0][0]
        nt = 1
        while (t0 + nt < T and tile_meta[t0 + nt][0] == r
               and nt < MAX_CALL // P):
            nt += 1
        calls.append((r, t0, nt))
        t0 += nt

    # per-core idx (int16, region-relative) and dst_local (f32) arrays
    idx_cols = sum(cl[2] * P // 16 for cl in calls)
    idx_all = np.zeros((NCORES, 16, idx_cols), np.int16)
    dl_all = np.zeros((NCORES, P, T), np.float32)

    # tile-stream position of each (r, b) group
    pos = {}
    t = 0
    for r in range(NREG):
        for b in range(NB):
            pos[(r, b)] = t
            t += int(tiles_rb[r, b])

    for c in range(NCORES):
        stid, dsl, reg, blk = per_core[c]
        eidx = np.zeros((T, P), np.int64)
        dloc = np.zeros((T, P), np.int64)
        for ti, (r, _b, _f, _l) in enumerate(tile_meta):
            eidx[ti, :] = 2 * r * TSH + NLOC      # zero row inside region r
        key = reg * NB + blk
        order = np.argsort(key, kind="stable")
        ks = key[order]
        st_ids = stid[order]
        dls = dsl[order]
        uq, starts = np.unique(ks, return_index=True)
        starts = list(starts) + [len(ks)]
        for u_i, k in enumerate(uq):
            r, b = int(k) // NB, int(k) % NB
            lo, hi = starts[u_i], starts[u_i + 1]
            n = hi - lo
            ti = pos[(r, b)] + np.arange(n) // P
            lane = np.arange(n) % P
            eidx[ti, lane] = st_ids[lo:hi]
            dloc[ti, lane] = dls[lo:hi] % P
        dl_all[c] = dloc.T.astype(np.float32)
        col0 = 0
        for (r, t0c, nt) in calls:
            flat = (eidx[t0c:t0c + nt].reshape(-1) - r * RSPAN).astype(np.int16)
            ncol = nt * P // 16
            idx_all[c, :, col0:col0 + ncol] = flat.reshape(ncol, 16).T
            col0 += ncol

    struct = {
        "N": N, "NSH": NSH, "NLOC": NLOC, "NB": NB, "TSH": TSH, "TBL": TBL,
        "RSPAN": RSPAN, "tile_meta": tile_meta, "calls": calls, "T": T,
        "idx_cols": idx_cols,
    }
    per_core_data = {"idx": idx_all, "dstloc": dl_all}
    return struct, per_core_data, dinv, perms


def _host_tables(x1, W11, dinv, perms, struct):
    """layer-1 table g1 = (dinv*x1) @ W11 in permuted table order, padded."""
    NSH, TSH = struct["NSH"], struct["TSH"]
    g1 = (dinv[:, None] * np.asarray(x1, np.float32)) @ np.asarray(W11, np.float32)
    t1 = np.zeros((struct["TBL"], ELEM), np.float32)
    for c in range(NCORES):
        t1[c * TSH: c * TSH + NSH, :D] = g1[c * NSH + perms[c]]
    return t1


# --------------------------------------------------------------- device side

def _build_program(struct, fc_b_val):
    import concourse.bacc as bacc
    import concourse.mybir as mybir
    import concourse.tile as tile
    from concourse.library_config import mlp
    from concourse.masks import make_identity

    NB, TSH, TBL, RSPAN = (struct["NB"], struct["TSH"], struct["TBL"],
                           struct["RSPAN"])
    NLOC = struct["NLOC"]
    T = struct["T"]
    tile_meta = struct["tile_meta"]
    calls = struct["calls"]
    idx_cols = struct["idx_cols"]

    nc = bacc.Bacc(None, target_bir_lowering=False, num_swdge_queues=4)
    dt = mybir.dt

    t1 = nc.declare_dram_parameter("t1", [TBL, ELEM], dt.float32, isOutput=False)
    idx = nc.declare_dram_parameter("idx", [P, idx_cols], dt.int16, isOutput=False)
    dstloc = nc.declare_dram_parameter("dstloc", [P, T], dt.float32, isOutput=False)
    dinvb = nc.declare_dram_parameter("dinvb", [P, NB], dt.float32, isOutput=False)
    brep = nc.declare_dram_parameter("brep", [P, 3 * D], dt.float32, isOutput=False)
    w2 = nc.declare_dram_parameter("w2", [D, D], dt.float32, isOutput=False)
    w3 = nc.declare_dram_parameter("w3", [D, D], dt.float32, isOutput=False)
    fcw = nc.declare_dram_parameter("fcw", [D, 1], dt.float32, isOutput=False)
    iota = nc.declare_dram_parameter("iota", [P, P], dt.float32, isOutput=False)
    y = nc.declare_dram_parameter("y", [NLOC, 1], dt.float32, isOutput=True)

    g2_loc = nc.dram_tensor("g2_loc", [TSH, ELEM], dt.float32)
    g3_loc = nc.dram_tensor("g3_loc", [TSH, ELEM], dt.float32)
    t2_sh = nc.dram_tensor("t2_sh", [TBL, ELEM], dt.float32, addr_space="Shared")
    t3_sh = nc.dram_tensor("t3_sh", [TBL, ELEM], dt.float32, addr_space="Shared")

    rg = [list(range(NCORES))]
    CHUNKS = MAX_CALL // P

    with tile.TileContext(nc) as tc:
        with (
            tc.tile_pool(name="const", bufs=1) as cpool,
            tc.tile_pool(name="msg", bufs=3) as mpool,
            tc.tile_pool(name="sel", bufs=6) as spool,
            tc.tile_pool(name="accs", bufs=1) as accpool,
            tc.tile_pool(name="ep", bufs=2) as epool,
            tc.tile_pool(name="gp", bufs=4, space="PSUM") as gpool,
            tc.tile_pool(name="eppsum", bufs=1, space="PSUM") as eppool,
        ):
            nc.gpsimd.load_library(mlp)
            idx_sb = cpool.tile([P, idx_cols], dt.int16)
            dl_sb = cpool.tile([P, T], dt.float32)
            dinv_sb = cpool.tile([P, NB], dt.float32)
            brep_sb = cpool.tile([P, 3 * D], dt.float32)
            w2_sb = cpool.tile([D, D], dt.float32)
            w3_sb = cpool.tile([D, D], dt.float32)
            fcw_sb = cpool.tile([D, 1], dt.float32)
            iota_sb = cpool.tile([P, P], dt.float32)
            ident = cpool.tile([P, P], dt.float32)
            zpad = cpool.tile([16, ELEM], dt.float32)

            nc.sync.dma_start(out=idx_sb[:], in_=idx[:])
            nc.sync.dma_start(out=dl_sb[:], in_=dstloc[:])
            nc.sync.dma_start(out=dinv_sb[:], in_=dinvb[:])
            nc.sync.dma_start(out=brep_sb[:], in_=brep[:])
            nc.sync.dma_start(out=w2_sb[:], in_=w2[:])
            nc.sync.dma_start(out=w3_sb[:], in_=w3[:])
            nc.sync.dma_start(out=fcw_sb[:], in_=fcw[:])
            nc.sync.dma_start(out=iota_sb[:], in_=iota[:])
            make_identity(nc, ident[:])
            nc.vector.memset(zpad[:], 0.0)
            nc.sync.dma_start(out=g2_loc[NLOC:TSH, :], in_=zpad[:])
            nc.sync.dma_start(out=g3_loc[NLOC:TSH, :], in_=zpad[:])
            tc.strict_bb_all_engine_barrier()

            tables = [t1, t2_sh, t3_sh]
            gouts = [g2_loc, g3_loc, None]
            wnext = [w2_sb, w3_sb, None]

            for L in range(3):
                table = tables[L]
                acc = accpool.tile([P, NB * D], dt.float32,
                                   name=f"acc{L}", tag="acc")
                gp = None
                icol = 0
                for ci, (r, t0c, nt) in enumerate(calls):
                    nidx = nt * P
                    ncol = nidx // 16
                    msg = mpool.tile([P, CHUNKS * ELEM], dt.float32,
                                     name=f"msg{L}_{ci}", tag="msg")
                    nc.gpsimd.dma_gather(
                        msg[:, : nt * ELEM].rearrange("p (c e) -> p c e", e=ELEM),
                        table[r * RSPAN:(r + 1) * RSPAN, :],
                        idx_sb[:, icol:icol + ncol],
                        nidx, nidx, ELEM,
                        single_packet=False, queue_num=ci % 4)
                    icol += ncol
                    msg3 = msg[:].rearrange("p (c e) -> p c e", e=ELEM)
                    # selection matrices for this call, 4 tiles per DVE op
                    sels = []
                    for q0 in range(0, nt, 4):
                        qn = min(4, nt - q0)
                        sel = spool.tile([P, 4 * P], dt.float32,
                                         name=f"sel{L}_{t0c + q0}", tag="sel")
                        tq = t0c + q0
                        nc.vector.tensor_tensor(
                            out=sel[:, : qn * P].rearrange(
                                "p (q j) -> p q j", q=qn),
                            in0=dl_sb[:, tq:tq + qn].unsqueeze(2).to_broadcast(
                                [P, qn, P]),
                            in1=iota_sb[:].unsqueeze(1).to_broadcast(
                                [P, qn, P]),
                            op=mybir.AluOpType.is_equal)
                        sels.append(sel)
                    for c in range(nt):
                        t_glob = t0c + c
                        _r, b, gfst, glst = tile_meta[t_glob]
                        sel = sels[c // 4]
                        if gfst:
                            gp = gpool.tile([P, D], dt.float32,
                                            name=f"gp{L}_{t_glob}", tag="gp")
                        nc.tensor.matmul(
                            out=gp[:],
                            lhsT=sel[:, (c % 4) * P:(c % 4 + 1) * P],
                            rhs=msg3[:, c, 0:D],
                            start=bool(gfst), stop=bool(glst),
                            skip_group_check=True)
                        if glst:
                            if _r == 0:
                                nc.vector.tensor_copy(
                                    out=acc[:, b * D:(b + 1) * D], in_=gp[:])
                            else:
                                nc.vector.tensor_tensor(
                                    out=acc[:, b * D:(b + 1) * D],
                                    in0=acc[:, b * D:(b + 1) * D],
                                    in1=gp[:], op=mybir.AluOpType.add)
                # ---- epilogue per block ----
                for b in range(NB):
                    xb = epool.tile([P, D], dt.float32, name=f"x{L}_{b}", tag="xb")
                    nc.vector.tensor_scalar(
                        out=xb[:], in0=acc[:, b * D:(b + 1) * D],
                        scalar1=dinv_sb[:, b:b + 1], scalar2=None,
                        op0=mybir.AluOpType.mult)
                    nc.vector.tensor_tensor(
                        out=xb[:], in0=xb[:], in1=brep_sb[:, L * D:(L + 1) * D],
                        op=mybir.AluOpType.add)
                    nc.vector.tensor_scalar(
                        out=xb[:], in0=xb[:], scalar1=0.0, scalar2=None,
                        op0=mybir.AluOpType.max)
                    xT = eppool.tile([D, P], dt.float32, name=f"xT{L}_{b}", tag="xT")
                    nc.tensor.transpose(out=xT[:], in_=xb[:], identity=ident[:])
                    xT_sb = epool.tile([D, P], dt.float32,
                                       name=f"xTs{L}_{b}", tag="xTs")
                    nc.scalar.copy(out=xT_sb[:], in_=xT[:])
                    if L < 2:
                        h = eppool.tile([P, D], dt.float32,
                                        name=f"h{L}_{b}", tag="h")
                        nc.tensor.matmul(out=h[:], lhsT=xT_sb[:],
                                         rhs=wnext[L][:], start=True, stop=True)
                        g_sb = epool.tile([P, D], dt.float32,
                                          name=f"g{L}_{b}", tag="g")
                        nc.vector.tensor_scalar(
                            out=g_sb[:], in0=h[:],
                            scalar1=dinv_sb[:, b:b + 1], scalar2=None,
                            op0=mybir.AluOpType.mult)
                        nc.sync.dma_start(
                            out=gouts[L][b * P:(b + 1) * P, 0:D], in_=g_sb[:])
                    else:
                        yp = eppool.tile([P, 1], dt.float32,
                                         name=f"yp{b}", tag="h")
                        nc.tensor.matmul(out=yp[:], lhsT=xT_sb[:],
                                         rhs=fcw_sb[:], start=True, stop=True)
                        y_sb = epool.tile([P, 1], dt.float32,
                                          name=f"ys{b}", tag="g")
                        nc.vector.tensor_scalar(
                            out=y_sb[:], in0=yp[:],
                            scalar1=float(fc_b_val), scalar2=None,
                            op0=mybir.AluOpType.add)
                        nc.sync.dma_start(out=y[b * P:(b + 1) * P, :], in_=y_sb[:])
                if L < 2:
                    tc.strict_bb_all_engine_barrier()
                    nc.gpsimd.collective_compute(
                        "AllGather", mybir.AluOpType.bypass,
                        replica_groups=rg,
                        ins=[gouts[L][:]],
                        outs=[tables[L + 1][:]])
                    tc.strict_bb_all_engine_barrier()
    nc.finalize()
    return nc


# ------------------------------------------------------------------- kernel

def kernel(x1, edge_index1, W11, b11, W12, b12, W13, b13, fc_w, fc_b):
    from concourse.bass_utils import run_bass_kernel_spmd

    x1 = np.asarray(x1, np.float32)
    edge_index = np.asarray(edge_index1)
    struct, pcd, dinv, perms = _preprocess(x1, edge_index)
    t1p = _host_tables(x1, W11, dinv, perms, struct)

    NB, NSH, NLOC = struct["NB"], struct["NSH"], struct["NLOC"]

    iota = np.tile(np.arange(P, dtype=np.float32)[None, :], (P, 1))
    brep = np.zeros((P, 3 * D), np.float32)
    brep[:, 0:D] = np.asarray(b11, np.float32)[None, :]
    brep[:, D:2 * D] = np.asarray(b12, np.float32)[None, :]
    brep[:, 2 * D:3 * D] = np.asarray(b13, np.float32)[None, :]

    in_maps = []
    for c in range(NCORES):
        dinv_loc = np.zeros(NLOC, np.float32)
        dinv_loc[:NSH] = dinv[c * NSH:(c + 1) * NSH][perms[c]]
        in_maps.append({
            "t1": t1p,
            "idx": np.tile(pcd["idx"][c], (8, 1)),
            "dstloc": pcd["dstloc"][c],
            "dinvb": dinv_loc.reshape(NB, P).T.copy(),
            "brep": brep,
            "w2": np.asarray(W12, np.float32),
            "w3": np.asarray(W13, np.float32),
            "fcw": np.asarray(fc_w, np.float32),
            "iota": iota,
        })

    nc = _build_program(struct, float(np.asarray(fc_b).reshape(-1)[0]))
    res = run_bass_kernel_spmd(nc, in_maps, core_ids=list(range(NCORES)))

    out = np.zeros((struct["N"], 1), np.float32)
    for c in range(NCORES):
        yc = res.results[c]["y"][:NSH, 0]
        out[c * NSH + perms[c], 0] = yc
    return out



# revision 21
# speedup vs baseline: 12290.5774x; 12290.5774x over previous
"""3-layer GCN (PyG GCNConv semantics) on 8 Trainium2 NeuronCores.

Strategy (graph/data parallel, dst-sharded, v2):
  - Tables are compact bf16 [TBL, 32] (TBL = 8 * 12544 rows). A gather
    "phase" f in 0..3 views rows f::4 so the 256B-stride int16-index
    dma_gather ucode reaches the whole table with 64B elements (32 bf16).
  - Host: degrees + symmetric norm, per-core degree-sorted permutation,
    edge lists grouped by (phase, dst-block-of-64), padded to a common
    tile structure across cores (SPMD), layer-1 table g1 = bf16(dinv *
    (x1 @ W11)).
  - Device per layer: relaxed dma_gather streams 64B bf16 rows (4 SWDGE
    queues); DVE builds per-tile one-hot selection in bf16 (2x perf mode
    via a materialized iota pattern); PE accumulates sel.T @ msg per
    dst-block directly in PSUM across all 4 phases (consume order is
    block-major while gather calls stay phase-major). Epilogue per 128-node
    block pair: ACT copies PSUM->SBUF with dinv scaling, DVE adds bias +
    relu (bf16), PE transpose + matmul with the next W, DVE scales by dinv
    into a staging buffer; one strided DMA writes the layer table shard.
  - AllGather exchanges compact bf16 table shards between layers.
  - Final layer: y = x4 @ fc_w + fc_b per block pair, fp32 output.
"""

import numpy as np
import ml_dtypes

P = 128
D = 32             # feature width
SLOT = 64          # dst-block width (one-hot width)
B = 16             # sel batch (tiles per DVE one-hot op)
NPHASE = 4         # table row phases (256B stride / 64B rows)
MAX_CT = 64        # tiles per gather call (64*128 = 8192 indices)
NCORES = 8
HP = 7             # table exchange pieces (pipelined AllGather)

bf16 = ml_dtypes.bfloat16


# ----------------------------------------------------------------- host side

def _preprocess(x1, edge_index):
    N = x1.shape[0]
    assert N % NCORES == 0
    NSH = N // NCORES                       # dst nodes per core
    NLOC = ((NSH + P - 1) // P) * P         # padded to 128
    NBLK = NLOC // SLOT
    NPAIR = NLOC // P
    TBL = NCORES * NLOC
    assert TBL % NPHASE == 0
    PSPAN = TBL // NPHASE
    assert PSPAN <= 32767

    src = np.asarray(edge_index[0], dtype=np.int64)
    dst = np.asarray(edge_index[1], dtype=np.int64)
    deg = np.bincount(dst, minlength=N).astype(np.float64) + 1.0
    dinv = (1.0 / np.sqrt(deg)).astype(np.float32)

    core_of = dst // NSH

    # table layout: HP exchange pieces; piece h holds slots [h*HS, (h+1)*HS)
    # of every core: row = piece*(TBL//HP) + core*HS + slot%HS
    assert NLOC % HP == 0 and (NLOC // HP) % P == 0
    HS = NLOC // HP
    TP = TBL // HP

    def slot2row(c, slot):
        h = slot // HS
        return h * TP + c * HS + slot % HS

    # per-core permutation (in-degree desc within shard), global->table map
    perms, invperms = [], []
    g2t = np.empty(N, np.int64)
    dcnt_all = np.bincount(dst, minlength=N) + 1
    for c in range(NCORES):
        cnt = dcnt_all[c * NSH:(c + 1) * NSH]
        perm = np.argsort(-cnt, kind="stable")
        inv = np.empty(NSH, np.int64)
        inv[perm] = np.arange(NSH)
        perms.append(perm)
        invperms.append(inv)
        g2t[c * NSH:(c + 1) * NSH] = slot2row(c, inv)

    # per-core edges (table row, dst slot) incl self-loops
    per_core = []
    for c in range(NCORES):
        m = core_of == c
        dslot = invperms[c][dst[m] - c * NSH]
        sl_d = invperms[c]
        row = np.concatenate([g2t[src[m]], g2t[np.arange(c * NSH, (c + 1) * NSH)]])
        slot = np.concatenate([dslot, sl_d])
        ph = row % NPHASE
        blk = slot // SLOT
        per_core.append((row, slot, ph, blk))

    # common tile structure over (phase, block)
    counts = np.zeros((NCORES, NPHASE, NBLK), np.int64)
    for c in range(NCORES):
        _, _, ph, blk = per_core[c]
        np.add.at(counts[c], (ph, blk), 1)
    tiles_fb = np.maximum((counts.max(axis=0) + P - 1) // P, 1)   # [NPHASE, NBLK]
    T = int(tiles_fb.sum())

    # call-order stream: phase-major, block order; pos of each group
    pos_fb = np.zeros((NPHASE, NBLK), np.int64)
    t = 0
    for f in range(NPHASE):
        for b in range(NBLK):
            pos_fb[f, b] = t
            t += int(tiles_fb[f, b])
    # gather calls: chunks of tiles within one phase
    calls = []          # (phase, t0, nt)
    for f in range(NPHASE):
        t0 = int(pos_fb[f, 0])
        tend = int(pos_fb[f + 1, 0]) if f + 1 < NPHASE else T
        while t0 < tend:
            nt = min(MAX_CT, tend - t0)
            calls.append((f, t0, nt))
            t0 += nt
    call_of_pos = np.zeros(T, np.int64)
    call_start = np.zeros(len(calls), np.int64)
    for k, (f, t0, nt) in enumerate(calls):
        call_of_pos[t0:t0 + nt] = k
        call_start[k] = t0

    # consume order: block-major; map consume tile -> call-order pos
    cons_pos = np.zeros(T, np.int64)
    ct = 0
    for b in range(NBLK):
        for f in range(NPHASE):
            nt = int(tiles_fb[f, b])
            cons_pos[ct:ct + nt] = pos_fb[f, b] + np.arange(nt)
            ct += nt
    assert ct == T

    # per-core idx (call order) and dl (consume order)
    idx_cols = T * P // 16
    idx_all = np.zeros((NCORES, P, idx_cols), np.int16)
    dl_all = np.full((NCORES, P, T), 255.0, np.float32)

    for c in range(NCORES):
        row, slot, ph, blk = per_core[c]
        eidx = np.zeros((T, P), np.int64)           # call-order, region-rel
        dloc = np.full((T, P), 255, np.int64)       # call-order
        key = ph * NBLK + blk
        order = np.argsort(key, kind="stable")
        ks = key[order]
        rows_s = row[order]
        slot_s = slot[order]
        uq, starts = np.unique(ks, return_index=True)
        starts = list(starts) + [len(ks)]
        for u_i, kk in enumerate(uq):
            f, b = int(kk) // NBLK, int(kk) % NBLK
            lo, hi = starts[u_i], starts[u_i + 1]
            n = hi - lo
            ti = pos_fb[f, b] + np.arange(n) // P
            lane = np.arange(n) % P
            eidx[ti, lane] = rows_s[lo:hi] // NPHASE
            dloc[ti, lane] = slot_s[lo:hi] % SLOT
        # idx in call order, wrapped 16 and replicated to 128 partitions
        w = eidx.reshape(T * P // 16, 16).T.astype(np.int16)     # [16, cols]
        idx_all[c] = np.tile(w, (8, 1))
        # dl in consume order
        dl_all[c] = dloc[cons_pos].T.astype(np.float32)

    struct = {
        "N": N, "NSH": NSH, "NLOC": NLOC, "NBLK": NBLK, "NPAIR": NPAIR,
        "TBL": TBL, "PSPAN": PSPAN, "T": T, "tiles_fb": tiles_fb,
        "calls": calls, "call_of_pos": call_of_pos, "call_start": call_start,
        "pos_fb": pos_fb, "idx_cols": idx_cols, "HS": HS, "TP": TP,
    }
    per_core_data = {"idx": idx_all, "dl": dl_all}
    return struct, per_core_data, dinv, perms, g2t


def _host_tables(x1, W11, dinv, g2t, struct):
    TBL = struct["TBL"]
    g1 = (dinv[:, None] * (np.asarray(x1, np.float32)
                           @ np.asarray(W11, np.float32).astype(bf16).astype(np.float32)))
    t1 = np.zeros((TBL, D), bf16)
    t1[g2t] = g1.astype(bf16)
    return t1


# --------------------------------------------------------------- device side

def _gather_small(g, out_ap, in_ap, idxs_ap, num_idxs, elem_size, elem_step,
                  queue_num=0):
    """dma_gather without the %256 elem-size restriction (non-transpose,
    DRAM source; the 256B requirement is transpose-only in the ucode)."""
    import concourse.mybir as mybir
    from concourse import ap_utils
    assert idxs_ap.dtype == mybir.dt.int16
    assert in_ap.dtype == out_ap.dtype
    assert ap_utils.ap_is_contiguous(in_ap.ap[1:])
    assert ap_utils.ap_is_contiguous(out_ap.ap[1:])
    assert ap_utils.ap_is_contiguous(idxs_ap.ap[1:])
    assert in_ap.ap[0][0] == elem_step
    assert in_ap.ap[-1][1] == out_ap.ap[-1][1] == elem_size
    stride_bytes = elem_step * mybir.dt.size(in_ap.dtype)
    stride_256, rem = divmod(stride_bytes, 256)
    assert rem == 0 and stride_256 < 256
    return g.add_instruction(mybir.InstDMAGatherAnt(
        name=g.bass.get_next_instruction_name(),
        ins=[*g.lower_ap_dma(in_ap, for_custom_bir_dma=True),
             g.lower_ap(idxs_ap),
             g.lower_val_access(g.to_reg(num_idxs))],
        outs=[g.lower_ap(out_ap)],
        transpose=False, num_idxs=num_idxs, elem_size=elem_size,
        stride_bytes_256=stride_256, gen_mode=0, single_packet=False,
        queue_num=queue_num, sbuf_tokens_per_rank=0,
        sbuf_free_dim_per_rank=0, sbuf_free_dim_pad_per_rank=0,
        sbuf_byte_offset=0))


def _build_program(struct, fc_b_val):
    import concourse.bacc as bacc
    import concourse.mybir as mybir
    import concourse.tile as tile
    from concourse.library_config import mlp
    from concourse.masks import make_identity

    NLOC, NBLK, NPAIR = struct["NLOC"], struct["NBLK"], struct["NPAIR"]
    TBL, PSPAN, T = struct["TBL"], struct["PSPAN"], struct["T"]
    tiles_fb = struct["tiles_fb"]
    calls = struct["calls"]
    call_of_pos = struct["call_of_pos"]
    call_start = struct["call_start"]
    pos_fb = struct["pos_fb"]
    idx_cols = struct["idx_cols"]

    nc = bacc.Bacc(None, target_bir_lowering=False, num_swdge_queues=4)
    dt = mybir.dt
    HS, TP = struct["HS"], struct["TP"]
    PPP = NPAIR // HP                   # pairs per exchange piece

    t1 = nc.declare_dram_parameter("t1", [TBL, D], dt.bfloat16, isOutput=False)
    idx = nc.declare_dram_parameter("idx", [P, idx_cols], dt.int16, isOutput=False)
    dlp = nc.declare_dram_parameter("dl", [P, T], dt.bfloat16, isOutput=False)
    dinvb = nc.declare_dram_parameter("dinvb", [P, NPAIR], dt.float32, isOutput=False)
    brep = nc.declare_dram_parameter("brep", [P, 3 * D], dt.bfloat16, isOutput=False)
    w2 = nc.declare_dram_parameter("w2", [D, D], dt.bfloat16, isOutput=False)
    w3 = nc.declare_dram_parameter("w3", [D, D], dt.bfloat16, isOutput=False)
    fcw = nc.declare_dram_parameter("fcw", [D, 1], dt.bfloat16, isOutput=False)
    iotaB = nc.declare_dram_parameter("iotaB", [P, SLOT * B], dt.bfloat16,
                                      isOutput=False)
    y = nc.declare_dram_parameter("y", [NLOC, 1], dt.float32, isOutput=True)

    g_loc = nc.dram_tensor("g_loc", [NLOC, D], dt.bfloat16)
    t2_sh = nc.dram_tensor("t2_sh", [TBL, D], dt.bfloat16, addr_space="Shared")
    t3_sh = nc.dram_tensor("t3_sh", [TBL, D], dt.bfloat16, addr_space="Shared")

    rg = [list(range(NCORES))]

    with tile.TileContext(nc) as tc:
        with (
            tc.tile_pool(name="const", bufs=1) as cpool,
            tc.tile_pool(name="msg", bufs=10) as mpool,
            tc.tile_pool(name="sel", bufs=4) as spool,
            tc.tile_pool(name="ep", bufs=2) as epool,
            tc.tile_pool(name="stage", bufs=1) as stpool,
            tc.tile_pool(name="gp", bufs=4, space="PSUM") as gpool,
            tc.tile_pool(name="eppsum", bufs=2, space="PSUM") as eppool,
        ):
            nc.gpsimd.load_library(mlp)
            idx_sb = cpool.tile([P, idx_cols], dt.int16)
            dl_sb = cpool.tile([P, T], dt.bfloat16)
            dinv_sb = cpool.tile([P, NPAIR], dt.float32)
            brep_sb = cpool.tile([P, 3 * D], dt.bfloat16)
            w2_sb = cpool.tile([D, D], dt.bfloat16)
            w3_sb = cpool.tile([D, D], dt.bfloat16)
            fcw_sb = cpool.tile([D, 1], dt.bfloat16)
            iota_sb = cpool.tile([P, SLOT * B], dt.bfloat16)
            ident = cpool.tile([P, P], dt.bfloat16)
            gbuf = stpool.tile([P, NPAIR, D], dt.bfloat16)
            ybuf = stpool.tile([P, NPAIR], dt.float32)

            nc.sync.dma_start(out=idx_sb[:], in_=idx[:])
            nc.sync.dma_start(out=dl_sb[:], in_=dlp[:])
            nc.sync.dma_start(out=dinv_sb[:], in_=dinvb[:])
            nc.sync.dma_start(out=brep_sb[:], in_=brep[:])
            nc.sync.dma_start(out=w2_sb[:], in_=w2[:])
            nc.sync.dma_start(out=w3_sb[:], in_=w3[:])
            nc.sync.dma_start(out=fcw_sb[:], in_=fcw[:])
            nc.sync.dma_start(out=iota_sb[:], in_=iotaB[:])
            make_identity(nc, ident[:])

            tables = [t1, t2_sh, t3_sh]
            wnext = [w2_sb, w3_sb, None]

            for L in range(3):
                table = tables[L]
                msg_tiles = [None] * len(calls)

                def ensure_call(k):
                    if msg_tiles[k] is not None:
                        return msg_tiles[k]
                    f, t0, nt = calls[k]
                    m = mpool.tile([P, MAX_CT, D], dt.bfloat16,
                                   name=f"msg{L}_{k}", tag="msg")
                    _gather_small(
                        nc.gpsimd,
                        m[:, 0:nt, :],
                        table[:].rearrange("(r q) d -> r (q d)", q=NPHASE)[
                            :, f * D:(f + 1) * D],
                        idx_sb[:, t0 * 8:(t0 + nt) * 8],
                        nt * P, D, NPHASE * D, queue_num=k % 4)
                    msg_tiles[k] = m
                    return m

                sel2 = None
                ct = 0
                for b in range(NBLK):
                    nt_b = int(tiles_fb[:, b].sum())
                    done = 0
                    gp = gpool.tile([SLOT, D], dt.float32,
                                    name=f"gp{L}_{b}", tag="gp")
                    for f in range(NPHASE):
                        pos0 = int(pos_fb[f, b])
                        for i in range(int(tiles_fb[f, b])):
                            p = pos0 + i
                            k = int(call_of_pos[p])
                            col = p - int(call_start[k])
                            m = ensure_call(k)
                            if ct % B == 0:
                                nb = min(B, T - ct)
                                sel2 = spool.tile([P, SLOT, B], dt.bfloat16,
                                                  name=f"sel{L}_{ct}", tag="sel")
                                nc.vector.tensor_tensor(
                                    out=sel2[:, :, 0:nb],
                                    in0=dl_sb[:, ct:ct + nb].unsqueeze(1)
                                        .to_broadcast([P, SLOT, nb]),
                                    in1=iota_sb[:].rearrange(
                                        "p (j u) -> p j u", u=B)[:, :, 0:nb],
                                    op=mybir.AluOpType.is_equal)
                            nc.tensor.matmul(
                                out=gp[:],
                                lhsT=sel2[:, :, ct % B],
                                rhs=m[:, col, :],
                                start=(done == 0), stop=(done == nt_b - 1),
                                skip_group_check=True)
                            done += 1
                            ct += 1
                    if b % 2 == 0:
                        gp_even = gp
                        continue
                    # ---- pair epilogue ----
                    q = b // 2
                    xb = epool.tile([P, D], dt.bfloat16, name=f"x{L}_{q}", tag="xb")
                    nc.scalar.activation(
                        out=xb[0:SLOT, :], in_=gp_even[:],
                        func=mybir.ActivationFunctionType.Copy,
                        scale=dinv_sb[0:SLOT, q:q + 1])
                    nc.scalar.activation(
                        out=xb[SLOT:P, :], in_=gp[:],
                        func=mybir.ActivationFunctionType.Copy,
                        scale=dinv_sb[SLOT:P, q:q + 1])
                    nc.vector.tensor_tensor(
                        out=xb[:], in0=xb[:], in1=brep_sb[:, L * D:(L + 1) * D],
                        op=mybir.AluOpType.add)
                    nc.vector.tensor_scalar(
                        out=xb[:], in0=xb[:], scalar1=0.0, scalar2=None,
                        op0=mybir.AluOpType.max)
                    xT = eppool.tile([D, P], dt.bfloat16, name=f"xT{L}_{q}", tag="xT")
                    nc.tensor.transpose(out=xT[:], in_=xb[:], identity=ident[:])
                    xT_sb = epool.tile([D, P], dt.bfloat16,
                                       name=f"xTs{L}_{q}", tag="xTs")
                    nc.scalar.copy(out=xT_sb[:], in_=xT[:])
                    if L < 2:
                        h = eppool.tile([P, D], dt.float32,
                                        name=f"h{L}_{q}", tag="h")
                        nc.tensor.matmul(out=h[:], lhsT=xT_sb[:],
                                         rhs=wnext[L][:], start=True, stop=True)
                        nc.vector.tensor_scalar(
                            out=gbuf[:, q, :], in0=h[:],
                            scalar1=dinv_sb[:, q:q + 1], scalar2=None,
                            op0=mybir.AluOpType.mult)
                        if (q + 1) % PPP == 0:
                            # exchange piece hx: slots [hx*HS, (hx+1)*HS)
                            hx = (q + 1) // PPP - 1
                            nc.sync.dma_start(
                                out=g_loc[hx * HS:(hx + 1) * HS, :].rearrange(
                                    "(q p) d -> p q d", p=P),
                                in_=gbuf[:, hx * PPP:(hx + 1) * PPP, :])
                            nc.gpsimd.collective_compute(
                                "AllGather", mybir.AluOpType.bypass,
                                replica_groups=rg,
                                ins=[g_loc[hx * HS:(hx + 1) * HS, :]],
                                outs=[tables[L + 1][hx * TP:(hx + 1) * TP, :]])
                    else:
                        yp = eppool.tile([P, 1], dt.float32,
                                         name=f"yp{q}", tag="h")
                        nc.tensor.matmul(out=yp[:], lhsT=xT_sb[:],
                                         rhs=fcw_sb[:], start=True, stop=True)
                        nc.vector.tensor_scalar(
                            out=ybuf[:, q:q + 1], in0=yp[:],
                            scalar1=float(fc_b_val), scalar2=None,
                            op0=mybir.AluOpType.add)
                if L < 2:
                    tc.strict_bb_all_engine_barrier()
                else:
                    nc.sync.dma_start(
                        out=y[:].rearrange("(q p) o -> p (q o)", p=P),
                        in_=ybuf[:])
    nc.finalize()
    return nc


# ------------------------------------------------------------------- kernel

def _prepare(x1, edge_index1, W11, b11, b12, b13, W12, W13, fc_w, fc_b):
    x1 = np.asarray(x1, np.float32)
    edge_index = np.asarray(edge_index1)
    struct, pcd, dinv, perms, g2t = _preprocess(x1, edge_index)
    t1p = _host_tables(x1, W11, dinv, g2t, struct)

    NLOC, NPAIR, NSH = struct["NLOC"], struct["NPAIR"], struct["NSH"]

    iotaB = np.zeros((P, SLOT, B), np.float32)
    for j in range(SLOT):
        iotaB[:, j, :] = j
    brep = np.zeros((P, 3 * D), np.float32)
    brep[:, 0:D] = np.asarray(b11, np.float32)[None, :]
    brep[:, D:2 * D] = np.asarray(b12, np.float32)[None, :]
    brep[:, 2 * D:3 * D] = np.asarray(b13, np.float32)[None, :]

    in_maps = []
    for c in range(NCORES):
        dinv_loc = np.zeros(NLOC, np.float32)
        dinv_loc[:NSH] = dinv[c * NSH:(c + 1) * NSH][perms[c]]
        in_maps.append({
            "t1": t1p,
            "idx": pcd["idx"][c],
            "dl": pcd["dl"][c].astype(bf16),
            "dinvb": dinv_loc.reshape(NPAIR, P).T.copy(),
            "brep": brep.astype(bf16),
            "w2": np.asarray(W12, np.float32).astype(bf16),
            "w3": np.asarray(W13, np.float32).astype(bf16),
            "fcw": np.asarray(fc_w, np.float32).astype(bf16),
            "iotaB": iotaB.reshape(P, SLOT * B).astype(bf16),
        })
    return struct, perms, in_maps


def kernel(x1, edge_index1, W11, b11, W12, b12, W13, b13, fc_w, fc_b):
    from concourse.bass_utils import run_bass_kernel_spmd

    struct, perms, in_maps = _prepare(
        x1, edge_index1, W11, b11, b12, b13, W12, W13, fc_w, fc_b)
    nc = _build_program(struct, float(np.asarray(fc_b).reshape(-1)[0]))
    res = run_bass_kernel_spmd(nc, in_maps, core_ids=list(range(NCORES)))

    NSH = struct["NSH"]
    out = np.zeros((struct["N"], 1), np.float32)
    for c in range(NCORES):
        yc = res.results[c]["y"][:NSH, 0]
        out[c * NSH + perms[c], 0] = yc
    return out


# revision 30
# speedup vs baseline: 12531.0714x; 1.0196x over previous
"""3-layer GCN (PyG GCNConv semantics) on 8 Trainium2 NeuronCores.

Strategy (graph/data parallel, dst-sharded, v2):
  - Tables are compact bf16 [TBL, 32] (TBL = 8 * 12544 rows). A gather
    "phase" f in 0..3 views rows f::4 so the 256B-stride int16-index
    dma_gather ucode reaches the whole table with 64B elements (32 bf16).
  - Host: degrees + symmetric norm, per-core degree-sorted permutation,
    edge lists grouped by (phase, dst-block-of-64), padded to a common
    tile structure across cores (SPMD), layer-1 table g1 = bf16(dinv *
    (x1 @ W11)).
  - Device per layer: relaxed dma_gather streams 64B bf16 rows (4 SWDGE
    queues); DVE builds per-tile one-hot selection in bf16 (2x perf mode
    via a materialized iota pattern); PE accumulates sel.T @ msg per
    dst-block directly in PSUM across all 4 phases (consume order is
    block-major while gather calls stay phase-major). Epilogue per 128-node
    block pair: ACT copies PSUM->SBUF with dinv scaling, DVE adds bias +
    relu (bf16), PE transpose + matmul with the next W, DVE scales by dinv
    into a staging buffer; one strided DMA writes the layer table shard.
  - AllGather exchanges compact bf16 table shards between layers.
  - Final layer: y = x4 @ fc_w + fc_b per block pair, fp32 output.
"""

import numpy as np
import ml_dtypes

P = 128
D = 32             # feature width
SLOT = 64          # dst-block width (one-hot width)
B = 8              # sel batch (tiles per DVE one-hot op)
NPHASE = 4         # table row phases (256B stride / 64B rows)
MAX_CT = 64        # tiles per gather call (64*128 = 8192 indices)
NCORES = 8
PIECE_PAIRS = [8, 14, 16, 16, 16, 14, 14]   # exchange piece sizes (128-node pairs)

bf16 = ml_dtypes.bfloat16


# ----------------------------------------------------------------- host side

def _preprocess(x1, edge_index):
    N = x1.shape[0]
    assert N % NCORES == 0
    NSH = N // NCORES                       # dst nodes per core
    NLOC = ((NSH + P - 1) // P) * P         # padded to 128
    NBLK = NLOC // SLOT
    NPAIR = NLOC // P
    TBL = NCORES * NLOC
    assert TBL % NPHASE == 0
    PSPAN = TBL // NPHASE
    assert PSPAN <= 32767

    src = np.asarray(edge_index[0], dtype=np.int64)
    dst = np.asarray(edge_index[1], dtype=np.int64)
    deg = np.bincount(dst, minlength=N).astype(np.float64) + 1.0
    dinv = (1.0 / np.sqrt(deg)).astype(np.float32)

    core_of = dst // NSH

    # table layout: exchange pieces of PIECE_PAIRS[h]*P slots; piece h holds
    # slots [HB[h], HB[h+1]) of every core:
    # row = NCORES*HB[h] + core*HSZ[h] + (slot - HB[h])
    assert sum(PIECE_PAIRS) == NPAIR
    HB = np.concatenate([[0], np.cumsum(PIECE_PAIRS)]) * P      # slot bounds
    HSZ = np.diff(HB)                                           # slots/piece
    TPOFF = NCORES * HB                                         # row bounds

    def slot2row(c, slot):
        h = np.searchsorted(HB, slot, side="right") - 1
        return TPOFF[h] + c * HSZ[h] + (slot - HB[h])

    # per-core permutation (in-degree desc within shard), global->table map
    perms, invperms = [], []
    g2t = np.empty(N, np.int64)
    dcnt_all = np.bincount(dst, minlength=N) + 1
    for c in range(NCORES):
        cnt = dcnt_all[c * NSH:(c + 1) * NSH]
        perm = np.argsort(-cnt, kind="stable")
        inv = np.empty(NSH, np.int64)
        inv[perm] = np.arange(NSH)
        perms.append(perm)
        invperms.append(inv)
        g2t[c * NSH:(c + 1) * NSH] = slot2row(c, inv)

    # per-core edges (table row, dst slot) incl self-loops
    per_core = []
    for c in range(NCORES):
        m = core_of == c
        dslot = invperms[c][dst[m] - c * NSH]
        sl_d = invperms[c]
        row = np.concatenate([g2t[src[m]], g2t[np.arange(c * NSH, (c + 1) * NSH)]])
        slot = np.concatenate([dslot, sl_d])
        ph = row % NPHASE
        blk = slot // SLOT
        per_core.append((row, slot, ph, blk))

    # common tile structure over (phase, block)
    counts = np.zeros((NCORES, NPHASE, NBLK), np.int64)
    for c in range(NCORES):
        _, _, ph, blk = per_core[c]
        np.add.at(counts[c], (ph, blk), 1)
    tiles_fb = np.maximum((counts.max(axis=0) + P - 1) // P, 1)   # [NPHASE, NBLK]
    T = int(tiles_fb.sum())

    # call-order stream: phase-major, block order; pos of each group
    pos_fb = np.zeros((NPHASE, NBLK), np.int64)
    t = 0
    for f in range(NPHASE):
        for b in range(NBLK):
            pos_fb[f, b] = t
            t += int(tiles_fb[f, b])
    # gather calls: chunks of tiles within one phase
    calls = []          # (phase, t0, nt)
    for f in range(NPHASE):
        t0 = int(pos_fb[f, 0])
        tend = int(pos_fb[f + 1, 0]) if f + 1 < NPHASE else T
        while t0 < tend:
            nt = min(MAX_CT, tend - t0)
            calls.append((f, t0, nt))
            t0 += nt
    call_of_pos = np.zeros(T, np.int64)
    call_start = np.zeros(len(calls), np.int64)
    for k, (f, t0, nt) in enumerate(calls):
        call_of_pos[t0:t0 + nt] = k
        call_start[k] = t0

    # consume order: block-major; map consume tile -> call-order pos
    cons_pos = np.zeros(T, np.int64)
    ct = 0
    for b in range(NBLK):
        for f in range(NPHASE):
            nt = int(tiles_fb[f, b])
            cons_pos[ct:ct + nt] = pos_fb[f, b] + np.arange(nt)
            ct += nt
    assert ct == T

    # per-core idx (call order) and dl (consume order)
    idx_cols = T * P // 16
    idx_all = np.zeros((NCORES, P, idx_cols), np.int16)
    dl_all = np.full((NCORES, P, T), 255.0, np.float32)

    for c in range(NCORES):
        row, slot, ph, blk = per_core[c]
        eidx = np.zeros((T, P), np.int64)           # call-order, region-rel
        dloc = np.full((T, P), 255, np.int64)       # call-order
        key = ph * NBLK + blk
        order = np.argsort(key, kind="stable")
        ks = key[order]
        rows_s = row[order]
        slot_s = slot[order]
        uq, starts = np.unique(ks, return_index=True)
        starts = list(starts) + [len(ks)]
        for u_i, kk in enumerate(uq):
            f, b = int(kk) // NBLK, int(kk) % NBLK
            lo, hi = starts[u_i], starts[u_i + 1]
            n = hi - lo
            ti = pos_fb[f, b] + np.arange(n) // P
            lane = np.arange(n) % P
            eidx[ti, lane] = rows_s[lo:hi] // NPHASE
            dloc[ti, lane] = slot_s[lo:hi] % SLOT
        # idx in call order, wrapped 16 and replicated to 128 partitions
        w = eidx.reshape(T * P // 16, 16).T.astype(np.int16)     # [16, cols]
        idx_all[c] = np.tile(w, (8, 1))
        # dl in consume order
        dl_all[c] = dloc[cons_pos].T.astype(np.float32)

    struct = {
        "N": N, "NSH": NSH, "NLOC": NLOC, "NBLK": NBLK, "NPAIR": NPAIR,
        "TBL": TBL, "PSPAN": PSPAN, "T": T, "tiles_fb": tiles_fb,
        "calls": calls, "call_of_pos": call_of_pos, "call_start": call_start,
        "pos_fb": pos_fb, "idx_cols": idx_cols,
        "HB": HB, "HSZ": HSZ, "TPOFF": TPOFF,
    }
    per_core_data = {"idx": idx_all, "dl": dl_all}
    return struct, per_core_data, dinv, perms, g2t


def _host_tables(x1, W11, dinv, g2t, struct):
    TBL = struct["TBL"]
    g1 = (dinv[:, None] * (np.asarray(x1, np.float32)
                           @ np.asarray(W11, np.float32).astype(bf16).astype(np.float32)))
    t1 = np.zeros((TBL, D), bf16)
    t1[g2t] = g1.astype(bf16)
    return t1


# --------------------------------------------------------------- device side

def _gather_small(g, out_ap, in_ap, idxs_ap, num_idxs, elem_size, elem_step,
                  queue_num=0):
    """dma_gather without the %256 elem-size restriction (non-transpose,
    DRAM source; the 256B requirement is transpose-only in the ucode)."""
    import concourse.mybir as mybir
    from concourse import ap_utils
    assert idxs_ap.dtype == mybir.dt.int16
    assert in_ap.dtype == out_ap.dtype
    assert ap_utils.ap_is_contiguous(in_ap.ap[1:])
    assert ap_utils.ap_is_contiguous(out_ap.ap[1:])
    assert ap_utils.ap_is_contiguous(idxs_ap.ap[1:])
    assert in_ap.ap[0][0] == elem_step
    assert in_ap.ap[-1][1] == out_ap.ap[-1][1] == elem_size
    stride_bytes = elem_step * mybir.dt.size(in_ap.dtype)
    stride_256, rem = divmod(stride_bytes, 256)
    assert rem == 0 and stride_256 < 256
    return g.add_instruction(mybir.InstDMAGatherAnt(
        name=g.bass.get_next_instruction_name(),
        ins=[*g.lower_ap_dma(in_ap, for_custom_bir_dma=True),
             g.lower_ap(idxs_ap),
             g.lower_val_access(g.to_reg(num_idxs))],
        outs=[g.lower_ap(out_ap)],
        transpose=False, num_idxs=num_idxs, elem_size=elem_size,
        stride_bytes_256=stride_256, gen_mode=0, single_packet=False,
        queue_num=queue_num, sbuf_tokens_per_rank=0,
        sbuf_free_dim_per_rank=0, sbuf_free_dim_pad_per_rank=0,
        sbuf_byte_offset=0))


def _build_program(struct, fc_b_val):
    import concourse.bacc as bacc
    import concourse.mybir as mybir
    import concourse.tile as tile
    from concourse.library_config import mlp
    from concourse.masks import make_identity

    NLOC, NBLK, NPAIR = struct["NLOC"], struct["NBLK"], struct["NPAIR"]
    TBL, PSPAN, T = struct["TBL"], struct["PSPAN"], struct["T"]
    tiles_fb = struct["tiles_fb"]
    calls = struct["calls"]
    call_of_pos = struct["call_of_pos"]
    call_start = struct["call_start"]
    pos_fb = struct["pos_fb"]
    idx_cols = struct["idx_cols"]

    nc = bacc.Bacc(None, target_bir_lowering=False, num_swdge_queues=4)
    dt = mybir.dt
    HB, HSZ, TPOFF = struct["HB"], struct["HSZ"], struct["TPOFF"]
    pair_end = list(np.cumsum(PIECE_PAIRS))      # pair index after each piece

    t1 = nc.declare_dram_parameter("t1", [TBL, D], dt.bfloat16, isOutput=False)
    idx = nc.declare_dram_parameter("idx", [P, idx_cols], dt.int16, isOutput=False)
    dlp = nc.declare_dram_parameter("dl", [P, T], dt.bfloat16, isOutput=False)
    dinvb = nc.declare_dram_parameter("dinvb", [P, NPAIR], dt.float32, isOutput=False)
    brep = nc.declare_dram_parameter("brep", [P, 3 * D], dt.bfloat16, isOutput=False)
    w2 = nc.declare_dram_parameter("w2", [D, D], dt.bfloat16, isOutput=False)
    w3 = nc.declare_dram_parameter("w3", [D, D], dt.bfloat16, isOutput=False)
    fcw = nc.declare_dram_parameter("fcw", [D, 1], dt.bfloat16, isOutput=False)
    iotaB = nc.declare_dram_parameter("iotaB", [P, SLOT * B], dt.bfloat16,
                                      isOutput=False)
    y = nc.declare_dram_parameter("y", [NLOC, 1], dt.float32, isOutput=True)

    g_loc = nc.dram_tensor("g_loc", [NLOC, D], dt.bfloat16)
    t2_sh = nc.dram_tensor("t2_sh", [TBL, D], dt.bfloat16, addr_space="Shared")
    t3_sh = nc.dram_tensor("t3_sh", [TBL, D], dt.bfloat16, addr_space="Shared")

    rg = [list(range(NCORES))]

    with tile.TileContext(nc) as tc:
        with (
            tc.tile_pool(name="const", bufs=1) as cpool,
            tc.tile_pool(name="msg", bufs=10) as mpool,
            tc.tile_pool(name="sel", bufs=6) as spool,
            tc.tile_pool(name="ep", bufs=2) as epool,
            tc.tile_pool(name="stage", bufs=1) as stpool,
            tc.tile_pool(name="gp", bufs=4, space="PSUM") as gpool,
            tc.tile_pool(name="eppsum", bufs=2, space="PSUM") as eppool,
        ):
            nc.gpsimd.load_library(mlp)
            idx_sb = cpool.tile([P, idx_cols], dt.int16)
            dl_sb = cpool.tile([P, T], dt.bfloat16)
            dinv_sb = cpool.tile([P, NPAIR], dt.float32)
            brep_sb = cpool.tile([P, 3 * D], dt.bfloat16)
            w2_sb = cpool.tile([D, D], dt.bfloat16)
            w3_sb = cpool.tile([D, D], dt.bfloat16)
            fcw_sb = cpool.tile([D, 1], dt.bfloat16)
            iota_sb = cpool.tile([P, SLOT * B], dt.bfloat16)
            ident = cpool.tile([P, P], dt.bfloat16)
            gbuf = stpool.tile([P, NPAIR, D], dt.bfloat16)
            ybuf = stpool.tile([P, NPAIR], dt.float32)

            for fch in range(NPHASE):
                c0 = int(pos_fb[fch, 0]) * 8
                c1 = int(pos_fb[fch + 1, 0]) * 8 if fch + 1 < NPHASE else idx_cols
                nc.sync.dma_start(out=idx_sb[:, c0:c1], in_=idx[:, c0:c1])
            nc.sync.dma_start(out=dl_sb[:], in_=dlp[:])
            nc.sync.dma_start(out=dinv_sb[:], in_=dinvb[:])
            nc.sync.dma_start(out=brep_sb[:], in_=brep[:])
            nc.sync.dma_start(out=w2_sb[:], in_=w2[:])
            nc.sync.dma_start(out=w3_sb[:], in_=w3[:])
            nc.sync.dma_start(out=fcw_sb[:], in_=fcw[:])
            nc.sync.dma_start(out=iota_sb[:], in_=iotaB[:])
            make_identity(nc, ident[:])

            tables = [t1, t2_sh, t3_sh]
            wnext = [w2_sb, w3_sb, None]

            for L in range(3):
                table = tables[L]
                msg_tiles = [None] * len(calls)

                def ensure_call(k):
                    if msg_tiles[k] is not None:
                        return msg_tiles[k]
                    f, t0, nt = calls[k]
                    m = mpool.tile([P, MAX_CT, D], dt.bfloat16,
                                   name=f"msg{L}_{k}", tag="msg")
                    _gather_small(
                        nc.gpsimd,
                        m[:, 0:nt, :],
                        table[:].rearrange("(r q) d -> r (q d)", q=NPHASE)[
                            :, f * D:(f + 1) * D],
                        idx_sb[:, t0 * 8:(t0 + nt) * 8],
                        nt * P, D, NPHASE * D, queue_num=k % 4)
                    msg_tiles[k] = m
                    return m

                sel2 = None
                ct = 0
                for b in range(NBLK):
                    nt_b = int(tiles_fb[:, b].sum())
                    done = 0
                    gp = gpool.tile([SLOT, D], dt.float32,
                                    name=f"gp{L}_{b}", tag="gp")
                    for f in range(NPHASE):
                        pos0 = int(pos_fb[f, b])
                        for i in range(int(tiles_fb[f, b])):
                            p = pos0 + i
                            k = int(call_of_pos[p])
                            col = p - int(call_start[k])
                            m = ensure_call(k)
                            if ct % B == 0:
                                nb = min(B, T - ct)
                                sel2 = spool.tile([P, SLOT, B], dt.bfloat16,
                                                  name=f"sel{L}_{ct}", tag="sel")
                                nc.vector.tensor_tensor(
                                    out=sel2[:, :, 0:nb],
                                    in0=dl_sb[:, ct:ct + nb].unsqueeze(1)
                                        .to_broadcast([P, SLOT, nb]),
                                    in1=iota_sb[:].rearrange(
                                        "p (j u) -> p j u", u=B)[:, :, 0:nb],
                                    op=mybir.AluOpType.is_equal)
                            nc.tensor.matmul(
                                out=gp[:],
                                lhsT=sel2[:, :, ct % B],
                                rhs=m[:, col, :],
                                start=(done == 0), stop=(done == nt_b - 1),
                                skip_group_check=True)
                            done += 1
                            ct += 1
                    if b % 2 == 0:
                        gp_even = gp
                        continue
                    # ---- pair epilogue ----
                    q = b // 2
                    xb = epool.tile([P, D], dt.bfloat16, name=f"x{L}_{q}", tag="xb")
                    nc.scalar.activation(
                        out=xb[0:SLOT, :], in_=gp_even[:],
                        func=mybir.ActivationFunctionType.Copy,
                        scale=dinv_sb[0:SLOT, q:q + 1])
                    nc.scalar.activation(
                        out=xb[SLOT:P, :], in_=gp[:],
                        func=mybir.ActivationFunctionType.Copy,
                        scale=dinv_sb[SLOT:P, q:q + 1])
                    nc.vector.tensor_tensor(
                        out=xb[:], in0=xb[:], in1=brep_sb[:, L * D:(L + 1) * D],
                        op=mybir.AluOpType.add)
                    nc.vector.tensor_scalar(
                        out=xb[:], in0=xb[:], scalar1=0.0, scalar2=None,
                        op0=mybir.AluOpType.max)
                    xT = eppool.tile([D, P], dt.bfloat16, name=f"xT{L}_{q}", tag="xT")
                    nc.tensor.transpose(out=xT[:], in_=xb[:], identity=ident[:])
                    xT_sb = epool.tile([D, P], dt.bfloat16,
                                       name=f"xTs{L}_{q}", tag="xTs")
                    nc.scalar.copy(out=xT_sb[:], in_=xT[:])
                    if L < 2:
                        h = eppool.tile([P, D], dt.float32,
                                        name=f"h{L}_{q}", tag="h")
                        nc.tensor.matmul(out=h[:], lhsT=xT_sb[:],
                                         rhs=wnext[L][:], start=True, stop=True)
                        nc.vector.tensor_scalar(
                            out=gbuf[:, q, :], in0=h[:],
                            scalar1=dinv_sb[:, q:q + 1], scalar2=None,
                            op0=mybir.AluOpType.mult)
                        if (q + 1) in pair_end:
                            # exchange piece hx: slots [HB[hx], HB[hx+1])
                            hx = pair_end.index(q + 1)
                            q0 = pair_end[hx - 1] if hx else 0
                            nc.sync.dma_start(
                                out=g_loc[HB[hx]:HB[hx + 1], :].rearrange(
                                    "(q p) d -> p q d", p=P),
                                in_=gbuf[:, q0:q + 1, :])
                            nc.gpsimd.collective_compute(
                                "AllGather", mybir.AluOpType.bypass,
                                replica_groups=rg,
                                ins=[g_loc[HB[hx]:HB[hx + 1], :]],
                                outs=[tables[L + 1][TPOFF[hx]:TPOFF[hx + 1], :]])
                    else:
                        yp = eppool.tile([P, 1], dt.float32,
                                         name=f"yp{q}", tag="h")
                        nc.tensor.matmul(out=yp[:], lhsT=xT_sb[:],
                                         rhs=fcw_sb[:], start=True, stop=True)
                        nc.vector.tensor_scalar(
                            out=ybuf[:, q:q + 1], in0=yp[:],
                            scalar1=float(fc_b_val), scalar2=None,
                            op0=mybir.AluOpType.add)
                if L < 2:
                    tc.strict_bb_all_engine_barrier()
                else:
                    nc.sync.dma_start(
                        out=y[:].rearrange("(q p) o -> p (q o)", p=P),
                        in_=ybuf[:])
    nc.finalize()
    return nc


# ------------------------------------------------------------------- kernel

def _prepare(x1, edge_index1, W11, b11, b12, b13, W12, W13, fc_w, fc_b):
    x1 = np.asarray(x1, np.float32)
    edge_index = np.asarray(edge_index1)
    struct, pcd, dinv, perms, g2t = _preprocess(x1, edge_index)
    t1p = _host_tables(x1, W11, dinv, g2t, struct)

    NLOC, NPAIR, NSH = struct["NLOC"], struct["NPAIR"], struct["NSH"]

    iotaB = np.zeros((P, SLOT, B), np.float32)
    for j in range(SLOT):
        iotaB[:, j, :] = j
    brep = np.zeros((P, 3 * D), np.float32)
    brep[:, 0:D] = np.asarray(b11, np.float32)[None, :]
    brep[:, D:2 * D] = np.asarray(b12, np.float32)[None, :]
    brep[:, 2 * D:3 * D] = np.asarray(b13, np.float32)[None, :]

    in_maps = []
    for c in range(NCORES):
        dinv_loc = np.zeros(NLOC, np.float32)
        dinv_loc[:NSH] = dinv[c * NSH:(c + 1) * NSH][perms[c]]
        in_maps.append({
            "t1": t1p,
            "idx": pcd["idx"][c],
            "dl": pcd["dl"][c].astype(bf16),
            "dinvb": dinv_loc.reshape(NPAIR, P).T.copy(),
            "brep": brep.astype(bf16),
            "w2": np.asarray(W12, np.float32).astype(bf16),
            "w3": np.asarray(W13, np.float32).astype(bf16),
            "fcw": np.asarray(fc_w, np.float32).astype(bf16),
            "iotaB": iotaB.reshape(P, SLOT * B).astype(bf16),
        })
    return struct, perms, in_maps


def kernel(x1, edge_index1, W11, b11, W12, b12, W13, b13, fc_w, fc_b):
    from concourse.bass_utils import run_bass_kernel_spmd

    struct, perms, in_maps = _prepare(
        x1, edge_index1, W11, b11, b12, b13, W12, W13, fc_w, fc_b)
    nc = _build_program(struct, float(np.asarray(fc_b).reshape(-1)[0]))
    res = run_bass_kernel_spmd(nc, in_maps, core_ids=list(range(NCORES)))

    NSH = struct["NSH"]
    out = np.zeros((struct["N"], 1), np.float32)
    for c in range(NCORES):
        yc = res.results[c]["y"][:NSH, 0]
        out[c * NSH + perms[c], 0] = yc
    return out


# revision 31
# speedup vs baseline: 12891.6074x; 1.0288x over previous
"""3-layer GCN (PyG GCNConv semantics) on 8 Trainium2 NeuronCores.

Strategy (graph/data parallel, dst-sharded, v2):
  - Tables are compact bf16 [TBL, 32] (TBL = 8 * 12544 rows). A gather
    "phase" f in 0..3 views rows f::4 so the 256B-stride int16-index
    dma_gather ucode reaches the whole table with 64B elements (32 bf16).
  - Host: degrees + symmetric norm, per-core degree-sorted permutation,
    edge lists grouped by (phase, dst-block-of-64), padded to a common
    tile structure across cores (SPMD), layer-1 table g1 = bf16(dinv *
    (x1 @ W11)).
  - Device per layer: relaxed dma_gather streams 64B bf16 rows (4 SWDGE
    queues); DVE builds per-tile one-hot selection in bf16 (2x perf mode
    via a materialized iota pattern); PE accumulates sel.T @ msg per
    dst-block directly in PSUM across all 4 phases (consume order is
    block-major while gather calls stay phase-major). Epilogue per 128-node
    block pair: ACT copies PSUM->SBUF with dinv scaling, DVE adds bias +
    relu (bf16), PE transpose + matmul with the next W, DVE scales by dinv
    into a staging buffer; one strided DMA writes the layer table shard.
  - AllGather exchanges compact bf16 table shards between layers.
  - Final layer: y = x4 @ fc_w + fc_b per block pair, fp32 output.
"""

import numpy as np
import ml_dtypes

P = 128
D = 32             # feature width
SLOT = 64          # dst-block width (one-hot width)
B = 8              # sel batch (tiles per DVE one-hot op)
NPHASE = 4         # table row phases (256B stride / 64B rows)
MAX_CT = 64        # tiles per gather call (64*128 = 8192 indices)
NCORES = 8
PIECE_PAIRS = [6, 16, 24, 22, 30]   # exchange piece sizes (128-node pairs)

bf16 = ml_dtypes.bfloat16


# ----------------------------------------------------------------- host side

def _preprocess(x1, edge_index):
    N = x1.shape[0]
    assert N % NCORES == 0
    NSH = N // NCORES                       # dst nodes per core
    NLOC = ((NSH + P - 1) // P) * P         # padded to 128
    NBLK = NLOC // SLOT
    NPAIR = NLOC // P
    TBL = NCORES * NLOC
    assert TBL % NPHASE == 0
    PSPAN = TBL // NPHASE
    assert PSPAN <= 32767

    src = np.asarray(edge_index[0], dtype=np.int64)
    dst = np.asarray(edge_index[1], dtype=np.int64)
    deg = np.bincount(dst, minlength=N).astype(np.float64) + 1.0
    dinv = (1.0 / np.sqrt(deg)).astype(np.float32)

    core_of = dst // NSH

    # table layout: exchange pieces of PIECE_PAIRS[h]*P slots; piece h holds
    # slots [HB[h], HB[h+1]) of every core:
    # row = NCORES*HB[h] + core*HSZ[h] + (slot - HB[h])
    assert sum(PIECE_PAIRS) == NPAIR
    HB = np.concatenate([[0], np.cumsum(PIECE_PAIRS)]) * P      # slot bounds
    HSZ = np.diff(HB)                                           # slots/piece
    TPOFF = NCORES * HB                                         # row bounds

    def slot2row(c, slot):
        h = np.searchsorted(HB, slot, side="right") - 1
        return TPOFF[h] + c * HSZ[h] + (slot - HB[h])

    # per-core permutation (in-degree desc within shard), global->table map
    perms, invperms = [], []
    g2t = np.empty(N, np.int64)
    dcnt_all = np.bincount(dst, minlength=N) + 1
    for c in range(NCORES):
        cnt = dcnt_all[c * NSH:(c + 1) * NSH]
        perm = np.argsort(-cnt, kind="stable")
        inv = np.empty(NSH, np.int64)
        inv[perm] = np.arange(NSH)
        perms.append(perm)
        invperms.append(inv)
        g2t[c * NSH:(c + 1) * NSH] = slot2row(c, inv)

    # per-core edges (table row, dst slot) incl self-loops
    per_core = []
    for c in range(NCORES):
        m = core_of == c
        dslot = invperms[c][dst[m] - c * NSH]
        sl_d = invperms[c]
        row = np.concatenate([g2t[src[m]], g2t[np.arange(c * NSH, (c + 1) * NSH)]])
        slot = np.concatenate([dslot, sl_d])
        ph = row % NPHASE
        blk = slot // SLOT
        per_core.append((row, slot, ph, blk))

    # common tile structure over (phase, block)
    counts = np.zeros((NCORES, NPHASE, NBLK), np.int64)
    for c in range(NCORES):
        _, _, ph, blk = per_core[c]
        np.add.at(counts[c], (ph, blk), 1)
    tiles_fb = np.maximum((counts.max(axis=0) + P - 1) // P, 1)   # [NPHASE, NBLK]
    T = int(tiles_fb.sum())

    # call-order stream: phase-major, block order; pos of each group
    pos_fb = np.zeros((NPHASE, NBLK), np.int64)
    t = 0
    for f in range(NPHASE):
        for b in range(NBLK):
            pos_fb[f, b] = t
            t += int(tiles_fb[f, b])
    # gather calls: chunks of tiles within one phase
    calls = []          # (phase, t0, nt)
    for f in range(NPHASE):
        t0 = int(pos_fb[f, 0])
        tend = int(pos_fb[f + 1, 0]) if f + 1 < NPHASE else T
        while t0 < tend:
            nt = min(MAX_CT, tend - t0)
            calls.append((f, t0, nt))
            t0 += nt
    call_of_pos = np.zeros(T, np.int64)
    call_start = np.zeros(len(calls), np.int64)
    for k, (f, t0, nt) in enumerate(calls):
        call_of_pos[t0:t0 + nt] = k
        call_start[k] = t0

    # consume order: block-major; map consume tile -> call-order pos
    cons_pos = np.zeros(T, np.int64)
    ct = 0
    for b in range(NBLK):
        for f in range(NPHASE):
            nt = int(tiles_fb[f, b])
            cons_pos[ct:ct + nt] = pos_fb[f, b] + np.arange(nt)
            ct += nt
    assert ct == T

    # per-core idx (call order) and dl (consume order)
    idx_cols = T * P // 16
    idx_all = np.zeros((NCORES, P, idx_cols), np.int16)
    dl_all = np.full((NCORES, P, T), 255.0, np.float32)

    for c in range(NCORES):
        row, slot, ph, blk = per_core[c]
        eidx = np.zeros((T, P), np.int64)           # call-order, region-rel
        dloc = np.full((T, P), 255, np.int64)       # call-order
        key = ph * NBLK + blk
        order = np.argsort(key, kind="stable")
        ks = key[order]
        rows_s = row[order]
        slot_s = slot[order]
        uq, starts = np.unique(ks, return_index=True)
        starts = list(starts) + [len(ks)]
        for u_i, kk in enumerate(uq):
            f, b = int(kk) // NBLK, int(kk) % NBLK
            lo, hi = starts[u_i], starts[u_i + 1]
            n = hi - lo
            ti = pos_fb[f, b] + np.arange(n) // P
            lane = np.arange(n) % P
            eidx[ti, lane] = rows_s[lo:hi] // NPHASE
            dloc[ti, lane] = slot_s[lo:hi] % SLOT
        # idx in call order, wrapped 16 and replicated to 128 partitions
        w = eidx.reshape(T * P // 16, 16).T.astype(np.int16)     # [16, cols]
        idx_all[c] = np.tile(w, (8, 1))
        # dl in consume order
        dl_all[c] = dloc[cons_pos].T.astype(np.float32)

    struct = {
        "N": N, "NSH": NSH, "NLOC": NLOC, "NBLK": NBLK, "NPAIR": NPAIR,
        "TBL": TBL, "PSPAN": PSPAN, "T": T, "tiles_fb": tiles_fb,
        "calls": calls, "call_of_pos": call_of_pos, "call_start": call_start,
        "pos_fb": pos_fb, "idx_cols": idx_cols,
        "HB": HB, "HSZ": HSZ, "TPOFF": TPOFF,
    }
    per_core_data = {"idx": idx_all, "dl": dl_all}
    return struct, per_core_data, dinv, perms, g2t


def _host_tables(x1, W11, dinv, g2t, struct):
    TBL = struct["TBL"]
    g1 = (dinv[:, None] * (np.asarray(x1, np.float32)
                           @ np.asarray(W11, np.float32).astype(bf16).astype(np.float32)))
    t1 = np.zeros((TBL, D), bf16)
    t1[g2t] = g1.astype(bf16)
    return t1


# --------------------------------------------------------------- device side

def _gather_small(g, out_ap, in_ap, idxs_ap, num_idxs, elem_size, elem_step,
                  queue_num=0):
    """dma_gather without the %256 elem-size restriction (non-transpose,
    DRAM source; the 256B requirement is transpose-only in the ucode)."""
    import concourse.mybir as mybir
    from concourse import ap_utils
    assert idxs_ap.dtype == mybir.dt.int16
    assert in_ap.dtype == out_ap.dtype
    assert ap_utils.ap_is_contiguous(in_ap.ap[1:])
    assert ap_utils.ap_is_contiguous(out_ap.ap[1:])
    assert ap_utils.ap_is_contiguous(idxs_ap.ap[1:])
    assert in_ap.ap[0][0] == elem_step
    assert in_ap.ap[-1][1] == out_ap.ap[-1][1] == elem_size
    stride_bytes = elem_step * mybir.dt.size(in_ap.dtype)
    stride_256, rem = divmod(stride_bytes, 256)
    assert rem == 0 and stride_256 < 256
    return g.add_instruction(mybir.InstDMAGatherAnt(
        name=g.bass.get_next_instruction_name(),
        ins=[*g.lower_ap_dma(in_ap, for_custom_bir_dma=True),
             g.lower_ap(idxs_ap),
             g.lower_val_access(g.to_reg(num_idxs))],
        outs=[g.lower_ap(out_ap)],
        transpose=False, num_idxs=num_idxs, elem_size=elem_size,
        stride_bytes_256=stride_256, gen_mode=0, single_packet=False,
        queue_num=queue_num, sbuf_tokens_per_rank=0,
        sbuf_free_dim_per_rank=0, sbuf_free_dim_pad_per_rank=0,
        sbuf_byte_offset=0))


def _build_program(struct, fc_b_val):
    import concourse.bacc as bacc
    import concourse.mybir as mybir
    import concourse.tile as tile
    from concourse.library_config import mlp
    from concourse.masks import make_identity

    NLOC, NBLK, NPAIR = struct["NLOC"], struct["NBLK"], struct["NPAIR"]
    TBL, PSPAN, T = struct["TBL"], struct["PSPAN"], struct["T"]
    tiles_fb = struct["tiles_fb"]
    calls = struct["calls"]
    call_of_pos = struct["call_of_pos"]
    call_start = struct["call_start"]
    pos_fb = struct["pos_fb"]
    idx_cols = struct["idx_cols"]

    nc = bacc.Bacc(None, target_bir_lowering=False, num_swdge_queues=4)
    dt = mybir.dt
    HB, HSZ, TPOFF = struct["HB"], struct["HSZ"], struct["TPOFF"]
    pair_end = list(np.cumsum(PIECE_PAIRS))      # pair index after each piece

    t1 = nc.declare_dram_parameter("t1", [TBL, D], dt.bfloat16, isOutput=False)
    idx = nc.declare_dram_parameter("idx", [P, idx_cols], dt.int16, isOutput=False)
    dlp = nc.declare_dram_parameter("dl", [P, T], dt.bfloat16, isOutput=False)
    dinvb = nc.declare_dram_parameter("dinvb", [P, NPAIR], dt.float32, isOutput=False)
    brep = nc.declare_dram_parameter("brep", [P, 3 * D], dt.bfloat16, isOutput=False)
    w2 = nc.declare_dram_parameter("w2", [D, D], dt.bfloat16, isOutput=False)
    w3 = nc.declare_dram_parameter("w3", [D, D], dt.bfloat16, isOutput=False)
    fcw = nc.declare_dram_parameter("fcw", [D, 1], dt.bfloat16, isOutput=False)
    iotaB = nc.declare_dram_parameter("iotaB", [P, SLOT * B], dt.bfloat16,
                                      isOutput=False)
    y = nc.declare_dram_parameter("y", [NLOC, 1], dt.float32, isOutput=True)

    g_loc = nc.dram_tensor("g_loc", [NLOC, D], dt.bfloat16)
    t2_sh = nc.dram_tensor("t2_sh", [TBL, D], dt.bfloat16, addr_space="Shared")
    t3_sh = nc.dram_tensor("t3_sh", [TBL, D], dt.bfloat16, addr_space="Shared")

    rg = [list(range(NCORES))]

    with tile.TileContext(nc) as tc:
        with (
            tc.tile_pool(name="const", bufs=1) as cpool,
            tc.tile_pool(name="msg", bufs=10) as mpool,
            tc.tile_pool(name="sel", bufs=6) as spool,
            tc.tile_pool(name="ep", bufs=2) as epool,
            tc.tile_pool(name="stage", bufs=1) as stpool,
            tc.tile_pool(name="gp", bufs=4, space="PSUM") as gpool,
            tc.tile_pool(name="eppsum", bufs=2, space="PSUM") as eppool,
        ):
            nc.gpsimd.load_library(mlp)
            idx_sb = cpool.tile([P, idx_cols], dt.int16)
            dl_sb = cpool.tile([P, T], dt.bfloat16)
            dinv_sb = cpool.tile([P, NPAIR], dt.float32)
            brep_sb = cpool.tile([P, 3 * D], dt.bfloat16)
            w2_sb = cpool.tile([D, D], dt.bfloat16)
            w3_sb = cpool.tile([D, D], dt.bfloat16)
            fcw_sb = cpool.tile([D, 1], dt.bfloat16)
            iota_sb = cpool.tile([P, SLOT * B], dt.bfloat16)
            ident = cpool.tile([P, P], dt.bfloat16)
            gbuf = stpool.tile([P, NPAIR, D], dt.bfloat16)
            ybuf = stpool.tile([P, NPAIR], dt.float32)

            for fch in range(NPHASE):
                c0 = int(pos_fb[fch, 0]) * 8
                c1 = int(pos_fb[fch + 1, 0]) * 8 if fch + 1 < NPHASE else idx_cols
                nc.sync.dma_start(out=idx_sb[:, c0:c1], in_=idx[:, c0:c1])
            nc.sync.dma_start(out=dl_sb[:], in_=dlp[:])
            nc.sync.dma_start(out=dinv_sb[:], in_=dinvb[:])
            nc.sync.dma_start(out=brep_sb[:], in_=brep[:])
            nc.sync.dma_start(out=w2_sb[:], in_=w2[:])
            nc.sync.dma_start(out=w3_sb[:], in_=w3[:])
            nc.sync.dma_start(out=fcw_sb[:], in_=fcw[:])
            nc.sync.dma_start(out=iota_sb[:], in_=iotaB[:])
            make_identity(nc, ident[:])

            tables = [t1, t2_sh, t3_sh]
            wnext = [w2_sb, w3_sb, None]

            for L in range(3):
                table = tables[L]
                msg_tiles = [None] * len(calls)

                def ensure_call(k):
                    if msg_tiles[k] is not None:
                        return msg_tiles[k]
                    f, t0, nt = calls[k]
                    m = mpool.tile([P, MAX_CT, D], dt.bfloat16,
                                   name=f"msg{L}_{k}", tag="msg")
                    _gather_small(
                        nc.gpsimd,
                        m[:, 0:nt, :],
                        table[:].rearrange("(r q) d -> r (q d)", q=NPHASE)[
                            :, f * D:(f + 1) * D],
                        idx_sb[:, t0 * 8:(t0 + nt) * 8],
                        nt * P, D, NPHASE * D, queue_num=k % 4)
                    msg_tiles[k] = m
                    return m

                sel2 = None
                ct = 0
                for b in range(NBLK):
                    nt_b = int(tiles_fb[:, b].sum())
                    done = 0
                    gp = gpool.tile([SLOT, D], dt.float32,
                                    name=f"gp{L}_{b}", tag="gp")
                    for f in range(NPHASE):
                        pos0 = int(pos_fb[f, b])
                        for i in range(int(tiles_fb[f, b])):
                            p = pos0 + i
                            k = int(call_of_pos[p])
                            col = p - int(call_start[k])
                            m = ensure_call(k)
                            if ct % B == 0:
                                nb = min(B, T - ct)
                                sel2 = spool.tile([P, SLOT, B], dt.bfloat16,
                                                  name=f"sel{L}_{ct}", tag="sel")
                                nc.vector.tensor_tensor(
                                    out=sel2[:, :, 0:nb],
                                    in0=dl_sb[:, ct:ct + nb].unsqueeze(1)
                                        .to_broadcast([P, SLOT, nb]),
                                    in1=iota_sb[:].rearrange(
                                        "p (j u) -> p j u", u=B)[:, :, 0:nb],
                                    op=mybir.AluOpType.is_equal)
                            nc.tensor.matmul(
                                out=gp[:],
                                lhsT=sel2[:, :, ct % B],
                                rhs=m[:, col, :],
                                start=(done == 0), stop=(done == nt_b - 1),
                                skip_group_check=True)
                            done += 1
                            ct += 1
                    if b % 2 == 0:
                        gp_even = gp
                        continue
                    # ---- pair epilogue ----
                    q = b // 2
                    xb = epool.tile([P, D], dt.bfloat16, name=f"x{L}_{q}", tag="xb")
                    nc.scalar.activation(
                        out=xb[0:SLOT, :], in_=gp_even[:],
                        func=mybir.ActivationFunctionType.Copy,
                        scale=dinv_sb[0:SLOT, q:q + 1])
                    nc.scalar.activation(
                        out=xb[SLOT:P, :], in_=gp[:],
                        func=mybir.ActivationFunctionType.Copy,
                        scale=dinv_sb[SLOT:P, q:q + 1])
                    nc.vector.tensor_tensor(
                        out=xb[:], in0=xb[:], in1=brep_sb[:, L * D:(L + 1) * D],
                        op=mybir.AluOpType.add)
                    nc.vector.tensor_scalar(
                        out=xb[:], in0=xb[:], scalar1=0.0, scalar2=None,
                        op0=mybir.AluOpType.max)
                    xT = eppool.tile([D, P], dt.bfloat16, name=f"xT{L}_{q}", tag="xT")
                    nc.tensor.transpose(out=xT[:], in_=xb[:], identity=ident[:])
                    xT_sb = epool.tile([D, P], dt.bfloat16,
                                       name=f"xTs{L}_{q}", tag="xTs")
                    nc.scalar.copy(out=xT_sb[:], in_=xT[:])
                    if L < 2:
                        h = eppool.tile([P, D], dt.float32,
                                        name=f"h{L}_{q}", tag="h")
                        nc.tensor.matmul(out=h[:], lhsT=xT_sb[:],
                                         rhs=wnext[L][:], start=True, stop=True)
                        nc.vector.tensor_scalar(
                            out=gbuf[:, q, :], in0=h[:],
                            scalar1=dinv_sb[:, q:q + 1], scalar2=None,
                            op0=mybir.AluOpType.mult)
                        if (q + 1) in pair_end:
                            # exchange piece hx: slots [HB[hx], HB[hx+1])
                            hx = pair_end.index(q + 1)
                            q0 = pair_end[hx - 1] if hx else 0
                            nc.sync.dma_start(
                                out=g_loc[HB[hx]:HB[hx + 1], :].rearrange(
                                    "(q p) d -> p q d", p=P),
                                in_=gbuf[:, q0:q + 1, :])
                            nc.gpsimd.collective_compute(
                                "AllGather", mybir.AluOpType.bypass,
                                replica_groups=rg,
                                ins=[g_loc[HB[hx]:HB[hx + 1], :]],
                                outs=[tables[L + 1][TPOFF[hx]:TPOFF[hx + 1], :]])
                    else:
                        yp = eppool.tile([P, 1], dt.float32,
                                         name=f"yp{q}", tag="h")
                        nc.tensor.matmul(out=yp[:], lhsT=xT_sb[:],
                                         rhs=fcw_sb[:], start=True, stop=True)
                        nc.vector.tensor_scalar(
                            out=ybuf[:, q:q + 1], in0=yp[:],
                            scalar1=float(fc_b_val), scalar2=None,
                            op0=mybir.AluOpType.add)
                if L < 2:
                    tc.strict_bb_all_engine_barrier()
                else:
                    nc.sync.dma_start(
                        out=y[:].rearrange("(q p) o -> p (q o)", p=P),
                        in_=ybuf[:])
    nc.finalize()
    return nc


# ------------------------------------------------------------------- kernel

def _prepare(x1, edge_index1, W11, b11, b12, b13, W12, W13, fc_w, fc_b):
    x1 = np.asarray(x1, np.float32)
    edge_index = np.asarray(edge_index1)
    struct, pcd, dinv, perms, g2t = _preprocess(x1, edge_index)
    t1p = _host_tables(x1, W11, dinv, g2t, struct)

    NLOC, NPAIR, NSH = struct["NLOC"], struct["NPAIR"], struct["NSH"]

    iotaB = np.zeros((P, SLOT, B), np.float32)
    for j in range(SLOT):
        iotaB[:, j, :] = j
    brep = np.zeros((P, 3 * D), np.float32)
    brep[:, 0:D] = np.asarray(b11, np.float32)[None, :]
    brep[:, D:2 * D] = np.asarray(b12, np.float32)[None, :]
    brep[:, 2 * D:3 * D] = np.asarray(b13, np.float32)[None, :]

    in_maps = []
    for c in range(NCORES):
        dinv_loc = np.zeros(NLOC, np.float32)
        dinv_loc[:NSH] = dinv[c * NSH:(c + 1) * NSH][perms[c]]
        in_maps.append({
            "t1": t1p,
            "idx": pcd["idx"][c],
            "dl": pcd["dl"][c].astype(bf16),
            "dinvb": dinv_loc.reshape(NPAIR, P).T.copy(),
            "brep": brep.astype(bf16),
            "w2": np.asarray(W12, np.float32).astype(bf16),
            "w3": np.asarray(W13, np.float32).astype(bf16),
            "fcw": np.asarray(fc_w, np.float32).astype(bf16),
            "iotaB": iotaB.reshape(P, SLOT * B).astype(bf16),
        })
    return struct, perms, in_maps


def kernel(x1, edge_index1, W11, b11, W12, b12, W13, b13, fc_w, fc_b):
    from concourse.bass_utils import run_bass_kernel_spmd

    struct, perms, in_maps = _prepare(
        x1, edge_index1, W11, b11, b12, b13, W12, W13, fc_w, fc_b)
    nc = _build_program(struct, float(np.asarray(fc_b).reshape(-1)[0]))
    res = run_bass_kernel_spmd(nc, in_maps, core_ids=list(range(NCORES)))

    NSH = struct["NSH"]
    out = np.zeros((struct["N"], 1), np.float32)
    for c in range(NCORES):
        yc = res.results[c]["y"][:NSH, 0]
        out[c * NSH + perms[c], 0] = yc
    return out


# revision 35
# speedup vs baseline: 12896.9602x; 1.0004x over previous
"""3-layer GCN (PyG GCNConv semantics) on 8 Trainium2 NeuronCores.

Strategy (graph/data parallel, dst-sharded, v2):
  - Tables are compact bf16 [TBL, 32] (TBL = 8 * 12544 rows). A gather
    "phase" f in 0..3 views rows f::4 so the 256B-stride int16-index
    dma_gather ucode reaches the whole table with 64B elements (32 bf16).
  - Host: degrees + symmetric norm, per-core degree-sorted permutation,
    edge lists grouped by (phase, dst-block-of-64), padded to a common
    tile structure across cores (SPMD), layer-1 table g1 = bf16(dinv *
    (x1 @ W11)).
  - Device per layer: relaxed dma_gather streams 64B bf16 rows (4 SWDGE
    queues); DVE builds per-tile one-hot selection in bf16 (2x perf mode
    via a materialized iota pattern); PE accumulates sel.T @ msg per
    dst-block directly in PSUM across all 4 phases (consume order is
    block-major while gather calls stay phase-major). Epilogue per 128-node
    block pair: ACT copies PSUM->SBUF with dinv scaling, DVE adds bias +
    relu (bf16), PE transpose + matmul with the next W, DVE scales by dinv
    into a staging buffer; one strided DMA writes the layer table shard.
  - AllGather exchanges compact bf16 table shards between layers.
  - Final layer: y = x4 @ fc_w + fc_b per block pair, fp32 output.
"""

import numpy as np
import ml_dtypes

P = 128
D = 32             # feature width
SLOT = 64          # dst-block width (one-hot width)
B = 8              # sel batch (tiles per DVE one-hot op)
NPHASE = 4         # table row phases (256B stride / 64B rows)
MAX_CT = 64        # tiles per gather call (64*128 = 8192 indices)
NCORES = 8
PIECE_PAIRS = [6, 16, 24, 22, 30]   # exchange piece sizes (128-node pairs)
PIECE_ORDER = [0, 4, 3, 2, 1]       # processing order of pieces

bf16 = ml_dtypes.bfloat16


# ----------------------------------------------------------------- host side

def _preprocess(x1, edge_index):
    N = x1.shape[0]
    assert N % NCORES == 0
    NSH = N // NCORES                       # dst nodes per core
    NLOC = ((NSH + P - 1) // P) * P         # padded to 128
    NBLK = NLOC // SLOT
    NPAIR = NLOC // P
    TBL = NCORES * NLOC
    assert TBL % NPHASE == 0
    PSPAN = TBL // NPHASE
    assert PSPAN <= 32767

    src = np.asarray(edge_index[0], dtype=np.int64)
    dst = np.asarray(edge_index[1], dtype=np.int64)
    deg = np.bincount(dst, minlength=N).astype(np.float64) + 1.0
    dinv = (1.0 / np.sqrt(deg)).astype(np.float32)

    core_of = dst // NSH

    # table layout: exchange pieces of PIECE_PAIRS[h]*P slots; piece h holds
    # slots [HB[h], HB[h+1]) of every core:
    # row = NCORES*HB[h] + core*HSZ[h] + (slot - HB[h])
    assert sum(PIECE_PAIRS) == NPAIR
    HB = np.concatenate([[0], np.cumsum(PIECE_PAIRS)]) * P      # slot bounds
    HSZ = np.diff(HB)                                           # slots/piece
    TPOFF = NCORES * HB                                         # row bounds

    def slot2row(c, slot):
        h = np.searchsorted(HB, slot, side="right") - 1
        return TPOFF[h] + c * HSZ[h] + (slot - HB[h])

    # pair/block processing order (which piece's pairs are computed when);
    # tail exposure is the last-processed piece's collective
    piece_order = list(PIECE_ORDER)
    pbnd = np.concatenate([[0], np.cumsum(PIECE_PAIRS)])
    pair_order = np.concatenate(
        [np.arange(pbnd[h], pbnd[h + 1]) for h in piece_order])
    border = np.stack([2 * pair_order, 2 * pair_order + 1], 1).reshape(-1)

    # per-core permutation (in-degree desc within shard), global->table map
    perms, invperms = [], []
    g2t = np.empty(N, np.int64)
    dcnt_all = np.bincount(dst, minlength=N) + 1
    for c in range(NCORES):
        cnt = dcnt_all[c * NSH:(c + 1) * NSH]
        perm = np.argsort(-cnt, kind="stable")
        inv = np.empty(NSH, np.int64)
        inv[perm] = np.arange(NSH)
        perms.append(perm)
        invperms.append(inv)
        g2t[c * NSH:(c + 1) * NSH] = slot2row(c, inv)

    # per-core edges (table row, dst slot) incl self-loops
    per_core = []
    for c in range(NCORES):
        m = core_of == c
        dslot = invperms[c][dst[m] - c * NSH]
        sl_d = invperms[c]
        row = np.concatenate([g2t[src[m]], g2t[np.arange(c * NSH, (c + 1) * NSH)]])
        slot = np.concatenate([dslot, sl_d])
        ph = row % NPHASE
        blk = slot // SLOT
        per_core.append((row, slot, ph, blk))

    # common tile structure over (phase, block)
    counts = np.zeros((NCORES, NPHASE, NBLK), np.int64)
    for c in range(NCORES):
        _, _, ph, blk = per_core[c]
        np.add.at(counts[c], (ph, blk), 1)
    tiles_fb = np.maximum((counts.max(axis=0) + P - 1) // P, 1)   # [NPHASE, NBLK]
    T = int(tiles_fb.sum())

    # call-order stream: phase-major, block order; pos of each group
    pos_fb = np.zeros((NPHASE, NBLK), np.int64)
    t = 0
    for f in range(NPHASE):
        for b in border:
            pos_fb[f, b] = t
            t += int(tiles_fb[f, b])
    # gather calls: chunks of tiles within one phase
    phase_tiles = tiles_fb.sum(axis=1)
    phase_start = np.concatenate([[0], np.cumsum(phase_tiles)])
    calls = []          # (phase, t0, nt)
    for f in range(NPHASE):
        t0 = int(phase_start[f])
        tend = int(phase_start[f + 1])
        while t0 < tend:
            nt = min(MAX_CT, tend - t0)
            calls.append((f, t0, nt))
            t0 += nt
    call_of_pos = np.zeros(T, np.int64)
    call_start = np.zeros(len(calls), np.int64)
    for k, (f, t0, nt) in enumerate(calls):
        call_of_pos[t0:t0 + nt] = k
        call_start[k] = t0

    # consume order: block-major; map consume tile -> call-order pos
    cons_pos = np.zeros(T, np.int64)
    ct = 0
    for b in border:
        for f in range(NPHASE):
            nt = int(tiles_fb[f, b])
            cons_pos[ct:ct + nt] = pos_fb[f, b] + np.arange(nt)
            ct += nt
    assert ct == T

    # per-core idx (call order) and dl (consume order)
    idx_cols = T * P // 16
    idx_all = np.zeros((NCORES, P, idx_cols), np.int16)
    dl_all = np.full((NCORES, P, T), 255.0, np.float32)

    for c in range(NCORES):
        row, slot, ph, blk = per_core[c]
        eidx = np.zeros((T, P), np.int64)           # call-order, region-rel
        dloc = np.full((T, P), 255, np.int64)       # call-order
        key = ph * NBLK + blk
        order = np.argsort(key, kind="stable")
        ks = key[order]
        rows_s = row[order]
        slot_s = slot[order]
        uq, starts = np.unique(ks, return_index=True)
        starts = list(starts) + [len(ks)]
        for u_i, kk in enumerate(uq):
            f, b = int(kk) // NBLK, int(kk) % NBLK
            lo, hi = starts[u_i], starts[u_i + 1]
            n = hi - lo
            ti = pos_fb[f, b] + np.arange(n) // P
            lane = np.arange(n) % P
            eidx[ti, lane] = rows_s[lo:hi] // NPHASE
            dloc[ti, lane] = slot_s[lo:hi] % SLOT
        # idx in call order, wrapped 16 and replicated to 128 partitions
        w = eidx.reshape(T * P // 16, 16).T.astype(np.int16)     # [16, cols]
        idx_all[c] = np.tile(w, (8, 1))
        # dl in consume order
        dl_all[c] = dloc[cons_pos].T.astype(np.float32)

    struct = {
        "N": N, "NSH": NSH, "NLOC": NLOC, "NBLK": NBLK, "NPAIR": NPAIR,
        "TBL": TBL, "PSPAN": PSPAN, "T": T, "tiles_fb": tiles_fb,
        "calls": calls, "call_of_pos": call_of_pos, "call_start": call_start,
        "pos_fb": pos_fb, "idx_cols": idx_cols,
        "HB": HB, "HSZ": HSZ, "TPOFF": TPOFF,
        "piece_order": piece_order, "pair_order": pair_order, "border": border,
    }
    per_core_data = {"idx": idx_all, "dl": dl_all}
    return struct, per_core_data, dinv, perms, g2t


def _host_tables(x1, W11, dinv, g2t, struct):
    TBL = struct["TBL"]
    g1 = (dinv[:, None] * (np.asarray(x1, np.float32)
                           @ np.asarray(W11, np.float32).astype(bf16).astype(np.float32)))
    t1 = np.zeros((TBL, D), bf16)
    t1[g2t] = g1.astype(bf16)
    return t1


# --------------------------------------------------------------- device side

def _gather_small(g, out_ap, in_ap, idxs_ap, num_idxs, elem_size, elem_step,
                  queue_num=0):
    """dma_gather without the %256 elem-size restriction (non-transpose,
    DRAM source; the 256B requirement is transpose-only in the ucode)."""
    import concourse.mybir as mybir
    from concourse import ap_utils
    assert idxs_ap.dtype == mybir.dt.int16
    assert in_ap.dtype == out_ap.dtype
    assert ap_utils.ap_is_contiguous(in_ap.ap[1:])
    assert ap_utils.ap_is_contiguous(out_ap.ap[1:])
    assert ap_utils.ap_is_contiguous(idxs_ap.ap[1:])
    assert in_ap.ap[0][0] == elem_step
    assert in_ap.ap[-1][1] == out_ap.ap[-1][1] == elem_size
    stride_bytes = elem_step * mybir.dt.size(in_ap.dtype)
    stride_256, rem = divmod(stride_bytes, 256)
    assert rem == 0 and stride_256 < 256
    return g.add_instruction(mybir.InstDMAGatherAnt(
        name=g.bass.get_next_instruction_name(),
        ins=[*g.lower_ap_dma(in_ap, for_custom_bir_dma=True),
             g.lower_ap(idxs_ap),
             g.lower_val_access(g.to_reg(num_idxs))],
        outs=[g.lower_ap(out_ap)],
        transpose=False, num_idxs=num_idxs, elem_size=elem_size,
        stride_bytes_256=stride_256, gen_mode=0, single_packet=False,
        queue_num=queue_num, sbuf_tokens_per_rank=0,
        sbuf_free_dim_per_rank=0, sbuf_free_dim_pad_per_rank=0,
        sbuf_byte_offset=0))


def _build_program(struct, fc_b_val):
    import concourse.bacc as bacc
    import concourse.mybir as mybir
    import concourse.tile as tile
    from concourse.library_config import mlp
    from concourse.masks import make_identity

    NLOC, NBLK, NPAIR = struct["NLOC"], struct["NBLK"], struct["NPAIR"]
    TBL, PSPAN, T = struct["TBL"], struct["PSPAN"], struct["T"]
    tiles_fb = struct["tiles_fb"]
    calls = struct["calls"]
    call_of_pos = struct["call_of_pos"]
    call_start = struct["call_start"]
    pos_fb = struct["pos_fb"]
    idx_cols = struct["idx_cols"]

    nc = bacc.Bacc(None, target_bir_lowering=False, num_swdge_queues=4)
    dt = mybir.dt
    HB, HSZ, TPOFF = struct["HB"], struct["HSZ"], struct["TPOFF"]
    piece_order = struct["piece_order"]
    border = struct["border"]
    # after processing the k-th pair (in processing order), fire piece:
    fire_at = {}
    acc = 0
    for h in piece_order:
        acc += PIECE_PAIRS[h]
        fire_at[acc - 1] = h

    t1 = nc.declare_dram_parameter("t1", [TBL, D], dt.bfloat16, isOutput=False)
    idx = nc.declare_dram_parameter("idx", [P, idx_cols], dt.int16, isOutput=False)
    dlp = nc.declare_dram_parameter("dl", [P, T], dt.bfloat16, isOutput=False)
    dinvb = nc.declare_dram_parameter("dinvb", [P, NPAIR], dt.float32, isOutput=False)
    brep = nc.declare_dram_parameter("brep", [P, 3 * D], dt.bfloat16, isOutput=False)
    w2 = nc.declare_dram_parameter("w2", [D, D], dt.bfloat16, isOutput=False)
    w3 = nc.declare_dram_parameter("w3", [D, D], dt.bfloat16, isOutput=False)
    fcw = nc.declare_dram_parameter("fcw", [D, 1], dt.bfloat16, isOutput=False)
    iotaB = nc.declare_dram_parameter("iotaB", [P, SLOT * B], dt.bfloat16,
                                      isOutput=False)
    y = nc.declare_dram_parameter("y", [NLOC, 1], dt.float32, isOutput=True)

    g_loc = nc.dram_tensor("g_loc", [NLOC, D], dt.bfloat16)
    t2_sh = nc.dram_tensor("t2_sh", [TBL, D], dt.bfloat16, addr_space="Shared")
    t3_sh = nc.dram_tensor("t3_sh", [TBL, D], dt.bfloat16, addr_space="Shared")

    rg = [list(range(NCORES))]

    with tile.TileContext(nc) as tc:
        with (
            tc.tile_pool(name="const", bufs=1) as cpool,
            tc.tile_pool(name="msg", bufs=10) as mpool,
            tc.tile_pool(name="sel", bufs=6) as spool,
            tc.tile_pool(name="ep", bufs=2) as epool,
            tc.tile_pool(name="stage", bufs=1) as stpool,
            tc.tile_pool(name="gp", bufs=4, space="PSUM") as gpool,
            tc.tile_pool(name="eppsum", bufs=2, space="PSUM") as eppool,
        ):
            nc.gpsimd.load_library(mlp)
            idx_sb = cpool.tile([P, idx_cols], dt.int16)
            dl_sb = cpool.tile([P, T], dt.bfloat16)
            dinv_sb = cpool.tile([P, NPAIR], dt.float32)
            brep_sb = cpool.tile([P, 3 * D], dt.bfloat16)
            w2_sb = cpool.tile([D, D], dt.bfloat16)
            w3_sb = cpool.tile([D, D], dt.bfloat16)
            fcw_sb = cpool.tile([D, 1], dt.bfloat16)
            iota_sb = cpool.tile([P, SLOT * B], dt.bfloat16)
            ident = cpool.tile([P, P], dt.bfloat16)
            gbuf = stpool.tile([P, NPAIR, D], dt.bfloat16)
            ybuf = stpool.tile([P, NPAIR], dt.float32)

            for fch in range(NPHASE):
                c0 = int(pos_fb[fch, 0]) * 8
                c1 = int(pos_fb[fch + 1, 0]) * 8 if fch + 1 < NPHASE else idx_cols
                nc.sync.dma_start(out=idx_sb[:, c0:c1], in_=idx[:, c0:c1])
            nc.sync.dma_start(out=dl_sb[:], in_=dlp[:])
            nc.sync.dma_start(out=dinv_sb[:], in_=dinvb[:])
            nc.sync.dma_start(out=brep_sb[:], in_=brep[:])
            nc.sync.dma_start(out=w2_sb[:], in_=w2[:])
            nc.sync.dma_start(out=w3_sb[:], in_=w3[:])
            nc.sync.dma_start(out=fcw_sb[:], in_=fcw[:])
            nc.sync.dma_start(out=iota_sb[:], in_=iotaB[:])
            make_identity(nc, ident[:])

            tables = [t1, t2_sh, t3_sh]
            wnext = [w2_sb, w3_sb, None]

            for L in range(3):
                table = tables[L]
                msg_tiles = [None] * len(calls)

                def ensure_call(k):
                    if msg_tiles[k] is not None:
                        return msg_tiles[k]
                    f, t0, nt = calls[k]
                    m = mpool.tile([P, MAX_CT, D], dt.bfloat16,
                                   name=f"msg{L}_{k}", tag="msg")
                    _gather_small(
                        nc.gpsimd,
                        m[:, 0:nt, :],
                        table[:].rearrange("(r q) d -> r (q d)", q=NPHASE)[
                            :, f * D:(f + 1) * D],
                        idx_sb[:, t0 * 8:(t0 + nt) * 8],
                        nt * P, D, NPHASE * D, queue_num=k % 4)
                    msg_tiles[k] = m
                    return m

                sel2 = None
                ct = 0
                for bi, b in enumerate(border):
                    nt_b = int(tiles_fb[:, b].sum())
                    done = 0
                    gp = gpool.tile([SLOT, D], dt.float32,
                                    name=f"gp{L}_{b}", tag="gp")
                    for f in range(NPHASE):
                        pos0 = int(pos_fb[f, b])
                        for i in range(int(tiles_fb[f, b])):
                            p = pos0 + i
                            k = int(call_of_pos[p])
                            col = p - int(call_start[k])
                            m = ensure_call(k)
                            if ct % B == 0:
                                nb = min(B, T - ct)
                                sel2 = spool.tile([P, SLOT, B], dt.bfloat16,
                                                  name=f"sel{L}_{ct}", tag="sel")
                                nc.vector.tensor_tensor(
                                    out=sel2[:, :, 0:nb],
                                    in0=dl_sb[:, ct:ct + nb].unsqueeze(1)
                                        .to_broadcast([P, SLOT, nb]),
                                    in1=iota_sb[:].rearrange(
                                        "p (j u) -> p j u", u=B)[:, :, 0:nb],
                                    op=mybir.AluOpType.is_equal)
                            nc.tensor.matmul(
                                out=gp[:],
                                lhsT=sel2[:, :, ct % B],
                                rhs=m[:, col, :],
                                start=(done == 0), stop=(done == nt_b - 1),
                                skip_group_check=True)
                            done += 1
                            ct += 1
                    if bi % 2 == 0:
                        gp_even = gp
                        continue
                    # ---- pair epilogue ----
                    q = b // 2
                    xb = epool.tile([P, D], dt.bfloat16, name=f"x{L}_{q}", tag="xb")
                    nc.scalar.activation(
                        out=xb[0:SLOT, :], in_=gp_even[:],
                        func=mybir.ActivationFunctionType.Copy,
                        scale=dinv_sb[0:SLOT, q:q + 1])
                    nc.scalar.activation(
                        out=xb[SLOT:P, :], in_=gp[:],
                        func=mybir.ActivationFunctionType.Copy,
                        scale=dinv_sb[SLOT:P, q:q + 1])
                    nc.vector.tensor_tensor(
                        out=xb[:], in0=xb[:], in1=brep_sb[:, L * D:(L + 1) * D],
                        op=mybir.AluOpType.add)
                    nc.vector.tensor_scalar(
                        out=xb[:], in0=xb[:], scalar1=0.0, scalar2=None,
                        op0=mybir.AluOpType.max)
                    xT = eppool.tile([D, P], dt.bfloat16, name=f"xT{L}_{q}", tag="xT")
                    nc.tensor.transpose(out=xT[:], in_=xb[:], identity=ident[:])
                    xT_sb = epool.tile([D, P], dt.bfloat16,
                                       name=f"xTs{L}_{q}", tag="xTs")
                    nc.scalar.copy(out=xT_sb[:], in_=xT[:])
                    if L < 2:
                        h = eppool.tile([P, D], dt.float32,
                                        name=f"h{L}_{q}", tag="h")
                        nc.tensor.matmul(out=h[:], lhsT=xT_sb[:],
                                         rhs=wnext[L][:], start=True, stop=True)
                        nc.vector.tensor_scalar(
                            out=gbuf[:, q, :], in0=h[:],
                            scalar1=dinv_sb[:, q:q + 1], scalar2=None,
                            op0=mybir.AluOpType.mult)
                        if bi // 2 in fire_at:
                            # exchange piece hx: slots [HB[hx], HB[hx+1])
                            hx = fire_at[bi // 2]
                            q0 = int(HB[hx]) // P
                            q1 = int(HB[hx + 1]) // P
                            nc.sync.dma_start(
                                out=g_loc[HB[hx]:HB[hx + 1], :].rearrange(
                                    "(q p) d -> p q d", p=P),
                                in_=gbuf[:, q0:q1, :])
                            nc.gpsimd.collective_compute(
                                "AllGather", mybir.AluOpType.bypass,
                                replica_groups=rg,
                                ins=[g_loc[HB[hx]:HB[hx + 1], :]],
                                outs=[tables[L + 1][TPOFF[hx]:TPOFF[hx + 1], :]])
                    else:
                        yp = eppool.tile([P, 1], dt.float32,
                                         name=f"yp{q}", tag="h")
                        nc.tensor.matmul(out=yp[:], lhsT=xT_sb[:],
                                         rhs=fcw_sb[:], start=True, stop=True)
                        nc.vector.tensor_scalar(
                            out=ybuf[:, q:q + 1], in0=yp[:],
                            scalar1=float(fc_b_val), scalar2=None,
                            op0=mybir.AluOpType.add)
                if L < 2:
                    tc.strict_bb_all_engine_barrier()
                else:
                    nc.sync.dma_start(
                        out=y[:].rearrange("(q p) o -> p (q o)", p=P),
                        in_=ybuf[:])
    nc.finalize()
    return nc


# ------------------------------------------------------------------- kernel

def _prepare(x1, edge_index1, W11, b11, b12, b13, W12, W13, fc_w, fc_b):
    x1 = np.asarray(x1, np.float32)
    edge_index = np.asarray(edge_index1)
    struct, pcd, dinv, perms, g2t = _preprocess(x1, edge_index)
    t1p = _host_tables(x1, W11, dinv, g2t, struct)

    NLOC, NPAIR, NSH = struct["NLOC"], struct["NPAIR"], struct["NSH"]

    iotaB = np.zeros((P, SLOT, B), np.float32)
    for j in range(SLOT):
        iotaB[:, j, :] = j
    brep = np.zeros((P, 3 * D), np.float32)
    brep[:, 0:D] = np.asarray(b11, np.float32)[None, :]
    brep[:, D:2 * D] = np.asarray(b12, np.float32)[None, :]
    brep[:, 2 * D:3 * D] = np.asarray(b13, np.float32)[None, :]

    in_maps = []
    for c in range(NCORES):
        dinv_loc = np.zeros(NLOC, np.float32)
        dinv_loc[:NSH] = dinv[c * NSH:(c + 1) * NSH][perms[c]]
        in_maps.append({
            "t1": t1p,
            "idx": pcd["idx"][c],
            "dl": pcd["dl"][c].astype(bf16),
            "dinvb": dinv_loc.reshape(NPAIR, P).T.copy(),
            "brep": brep.astype(bf16),
            "w2": np.asarray(W12, np.float32).astype(bf16),
            "w3": np.asarray(W13, np.float32).astype(bf16),
            "fcw": np.asarray(fc_w, np.float32).astype(bf16),
            "iotaB": iotaB.reshape(P, SLOT * B).astype(bf16),
        })
    return struct, perms, in_maps


def kernel(x1, edge_index1, W11, b11, W12, b12, W13, b13, fc_w, fc_b):
    from concourse.bass_utils import run_bass_kernel_spmd

    struct, perms, in_maps = _prepare(
        x1, edge_index1, W11, b11, b12, b13, W12, W13, fc_w, fc_b)
    nc = _build_program(struct, float(np.asarray(fc_b).reshape(-1)[0]))
    res = run_bass_kernel_spmd(nc, in_maps, core_ids=list(range(NCORES)))

    NSH = struct["NSH"]
    out = np.zeros((struct["N"], 1), np.float32)
    for c in range(NCORES):
        yc = res.results[c]["y"][:NSH, 0]
        out[c * NSH + perms[c], 0] = yc
    return out
